# revision 19
# baseline (speedup 1.0000x reference)
"""CARAFE + MSGConv Trainium2 kernel (8 NeuronCores, spatial x batch sharding).

out[c, i, j] = sum_{p,q} W[5p+q, i, j] * Xpad[c, i//2 + p - 2, j//2 + q - 2]
 (CARAFE taps live at source resolution; identical for both subpixel parities).

Per core: one batch element (core//4) and a 16-source-row block (core%4).
The 25-tap reassembly runs on the TensorEngine as one K=120 matmul per
(row-pair, column-quarter) block:
  out[c, n] = sum_{(u,v)} X6T[(u,v), c] * B4[(u,v), n]
where B4 is a banded matrix of softmaxed W values built at runtime with
gpsimd local_scatter (per-partition index scatter) + a PE transpose; the
X side (X6T) is static data and comes pre-transposed from the host.

v3 restructure vs v2:
- one big input DMA (x | packa | repl) + xt + 4 small ones; ident moved
  into the early pack so the diag builds overlap the DMA window
- PE warm-up burst at t=0 (8 junk matmuls on a memset scratch) so HAM
  un-throttles to K=8/8 before the first real matmul
- stationary-weight reuse ordering: cv1/px run all matmuls sharing one
  lhsT back-to-back (banked PSUM accumulation groups interleaved)
- diag tiles: comp built by DVE, enc built by gpsimd local_scatter with
  an iota index (both overlap the head phase)
- backend coarsened: 4 repl matmuls land in one [128,400] PSUM bank ->
  one cast per t; 4 transposes land in one [120,512] bf16 bank -> one
  b4 copy per t; finals land in one [128,512] bank -> one stg copy per
  t into a persistent [128,4096] staging tile; 4 contiguous out DMAs
- output DRAM layout is t-major contiguous (no strided descriptors)
"""

import sys

sys.path.insert(0, "/opt/trn_rl_repo")

from contextlib import ExitStack

import ml_dtypes
import numpy as np

import concourse.bass as bass
import concourse.tile as tile
from concourse import bacc, library_config, mybir
from concourse.bass_utils import run_bass_kernel_spmd

BF16 = mybir.dt.bfloat16
F32 = mybir.dt.float32
I16 = mybir.dt.int16
AF = mybir.ActivationFunctionType
OP = mybir.AluOpType
AX = mybir.AxisListType
nbf = ml_dtypes.bfloat16

C = 128
H = W = 64
NCORES = 8
XR = 24          # X shard rows (16 + 4 halo each side)
XW = 68          # padded width for dw slabs only
NEG = -30.0      # additive pre-activation mask; SiLU(-30) ~= -2.8e-12

# packa column layout (bf16), all DMA'd in one shot with x
PA_CV1 = 0       # w_cv1_rep   [128, 128]
PA_ID = 128      # ident       [128, 128]
PA_ONES = 256    # ones row    [1, 128]
PA_PXA = 384     # w_pxA       [32, 64]
PA_PXB = 448     # w_pxB       [32, 64]
PA_ECV = 512     # w_ecv1_rep  [65, 128] (M groups at 0:50 and 64:114)
PA_EPA = 640     # w_epxA      [51, 100]
PA_EPB = 740     # w_epxB      [50, 100]
PA_EPBIAS = 840  # b_epx_row   [1, 100]
PA_EDW = 940     # w_edwp bf16 [128, 25] (for gpsimd diag builds)
PA_W = 966

PK1_W = PA_W + 512       # pk1: [packa | x[:, 0:512]]
PK2_W = 1536             # pk2: [x[:, 512:1536] | repl]

# comp dw tap split: DVE / Act / PE
CD_DVE = list(range(0, 7))
CD_ACT = list(range(7, 12))
CD_PE = list(range(12, 25))
# enc dw tap split
ED_DVE = list(range(0, 7))
ED_ACT = list(range(7, 11))
ED_PE = list(range(11, 25))


# ======================================================================
# host-side parameter prep
# ======================================================================

def _fold_1x1(w, s):
    return (w[:, :, 0, 0] * s[:, None]).T.copy()


def _dw_taps(w, s, k):
    ch = w.shape[0]
    out = np.zeros((ch, 25), np.float32)
    off = (5 - k) // 2
    for ty in range(k):
        for tx in range(k):
            out[:, 5 * (ty + off) + (tx + off)] = w[:, 0, ty, tx] * s
    return out


def _host_consts(inputs):
    d = {}
    w_cv1 = _fold_1x1(inputs["comp_cv1_w"], inputs["comp_cv1_s"])   # [128, 32]
    b_cv1 = inputs["comp_cv1_b"]                                    # [32]
    w3 = _dw_taps(inputs["comp_dw3_w"], inputs["comp_dw3_s"], 3)
    w5 = _dw_taps(inputs["comp_dw5_w"], inputs["comp_dw5_s"], 5)
    w_dwp = np.tile(np.concatenate([w3, w5], 0), (4, 1))            # [128, 25]
    b_dwp = np.tile(
        np.concatenate([inputs["comp_dw3_b"], inputs["comp_dw5_b"]]), 4
    ).reshape(128, 1)
    w_px = _fold_1x1(inputs["comp_px_w"], inputs["comp_px_s"])      # [64, 64]
    b_px = inputs["comp_px_b"].reshape(64, 1)
    we = _fold_1x1(inputs["enc_cv1_w"], inputs["enc_cv1_s"])        # [64, 50]
    w_ecv1 = np.concatenate([we, np.ones((1, 50), np.float32)], 0)  # [65, 50]
    b_ecv1 = inputs["enc_cv1_b"]                                    # [50]
    e3 = _dw_taps(inputs["enc_dw3_w"], inputs["enc_dw3_s"], 3)
    e5 = _dw_taps(inputs["enc_dw5_w"], inputs["enc_dw5_s"], 5)
    w_edw50 = np.concatenate([e3, e5], 0)                           # [50, 25]
    b_edw50 = np.concatenate(
        [inputs["enc_dw3_b"], inputs["enc_dw5_b"]]
    ).reshape(50, 1)
    wpx_e = _fold_1x1(inputs["enc_px_w"], inputs["enc_px_s"])       # [100, 100]
    b_epx = inputs["enc_px_b"].reshape(1, 100)

    w_edwp = np.zeros((128, 25), np.float32)
    w_edwp[0:50] = w_edw50
    w_edwp[64:114] = w_edw50

    pa = np.zeros((128, PA_W), np.float32)
    pa[:, PA_CV1 : PA_CV1 + 128] = np.tile(w_cv1, (1, 4))
    pa[:, PA_ID : PA_ID + 128] = np.eye(128)
    pa[0:1, PA_ONES : PA_ONES + 128] = 1.0
    pa[0:32, PA_PXA : PA_PXA + 64] = w_px[0:32]
    pa[0:32, PA_PXB : PA_PXB + 64] = w_px[32:64]
    pa[0:65, PA_ECV : PA_ECV + 50] = w_ecv1
    pa[0:65, PA_ECV + 64 : PA_ECV + 114] = w_ecv1
    pa[0:50, PA_EPA : PA_EPA + 100] = wpx_e[0:50]
    pa[50:51, PA_EPA : PA_EPA + 100] = b_epx
    pa[0:50, PA_EPB : PA_EPB + 100] = wpx_e[50:100]
    pa[0:1, PA_EPBIAS : PA_EPBIAS + 100] = b_epx
    pa[:, PA_EDW : PA_EDW + 25] = w_edwp
    d["_pa"] = pa.astype(nbf)

    pb = np.zeros((128, 56), np.float32)
    pb[:, 0:25] = w_dwp
    pb[:, 25:26] = b_dwp
    pb[0:50, 26:51] = w_edw50
    pb[64:114, 26:51] = w_edw50
    pb[0:50, 51:52] = b_edw50
    pb[64:114, 51:52] = b_edw50
    pb[:, 52:53] = np.tile(b_cv1, 4).reshape(128, 1)
    pb[0:64, 53:54] = b_px
    pb[0:50, 54:55] = b_ecv1.reshape(50, 1)
    pb[64:114, 54:55] = b_ecv1.reshape(50, 1)
    # SiLU(1.2784645) = 1: the ecv1 act writes a ones row at partition 50,
    # which carries the enc-px bias via K=51 on the e1-part matmul
    pb[50:51, 54:55] = 1.2784645427610737
    d["packb"] = pb

    # repl [128, 4*128]: lhsT for the W row-replication matmul
    # n raster within a block: n = 32*(2*yl+dy) + (2*xl+dx)
    rp = np.zeros((128, 512), np.float32)
    for jb in range(4):
        for n in range(128):
            rho, j = divmod(n, 32)
            yl, xl = rho // 2, j // 2
            rp[64 * yl + 16 * jb + xl, 128 * jb + n] = 1.0
    d["_repl"] = rp.astype(nbf)

    # sidx [128, 400+2] int16; cols 0:400 = scatter map (4 blocks per t),
    # cols 400:402 = iota idx for the gpsimd diag builds.
    si = np.full((128, 402), -1, np.int16)
    for n in range(128):
        rho, j = divmod(n, 32)
        yl, dy = divmod(rho, 2)
        xl, dx = divmod(j, 2)
        sn = 2 * dy + dx
        for jb in range(4):
            for cp in range(100):
                sc, k = divmod(cp, 25)
                if sc != sn:
                    continue
                p, q = divmod(k, 5)
                if not (0 <= 16 * jb + xl + q - 2 < 64):
                    continue
                si[n, 100 * jb + cp] = 120 * jb + 20 * (yl + p) + (xl + q)
        si[n, 400] = n
    d["sidx"] = si
    return d


def _host_shard(X, core):
    b, ri = divmod(core, 4)
    r0 = 16 * ri - 4
    xs = np.zeros((C, XR, W), np.float32)
    lo, hi = max(0, r0), min(H, r0 + XR)
    xs[:, lo - r0 : hi - r0, :] = X[b, :, lo:hi, :]
    mrow = np.zeros((1, XR, W), np.float32)
    for r in range(XR):
        if not (0 <= r0 + r < H):
            mrow[0, r, :] = NEG
    emask = np.zeros((1, 20, W), np.float32)
    for r in range(20):
        if not (0 <= (16 * ri - 2) + r < H):
            emask[0, r, :] = NEG
    xsb = xs.astype(nbf)
    # pre-transposed X slabs, one [120, 128] per block (column-padded)
    xsp = np.zeros((C, XR, XW), nbf)
    xsp[:, :, 2 : 2 + W] = xsb
    xt = np.zeros((120, 32 * 128), nbf)
    for B in range(32):
        t, jb = divmod(B, 4)
        slab = xsp[:, 2 * t + 2 : 2 * t + 8, 16 * jb : 16 * jb + 20]
        xt[:, 128 * B : 128 * B + 128] = slab.reshape(C, 120).T
    return (
        xsb.reshape(C, XR * W),
        mrow.reshape(1, XR * W).astype(nbf),
        emask.reshape(1, 20 * W).astype(nbf),
        xt,
    )


# ======================================================================
# device kernel
# ======================================================================

def build_kernel():
    nc = bacc.Bacc(
        "TRN2",
        target_bir_lowering=False,
        debug=False,
        enable_asserts=False,
        num_devices=NCORES,
    )

    def din(name, shape, dt):
        return nc.dram_tensor(name, list(shape), dt, kind="ExternalInput").ap()

    pk1_d = din("pk1", (128, PK1_W), BF16)
    pk2_d = din("pk2", (128, PK2_W), BF16)
    xt_d = din("xt", (120, 32 * 128), BF16)
    mrow_d = din("mrow", (1, XR * W), BF16)
    emask_d = din("emask", (1, 20 * W), BF16)
    packb_d = din("packb", (128, 56), F32)
    sidx_d = din("sidx", (128, 402), I16)
    # out layout: [c, (t, r, col)] -- t-major contiguous
    out_d = nc.dram_tensor("out", [128, 32 * 128], BF16, kind="ExternalOutput").ap()

    with tile.TileContext(nc) as tc, ExitStack() as ctx:
        cpool = ctx.enter_context(tc.tile_pool(name="consts", bufs=1))
        work = ctx.enter_context(tc.tile_pool(name="work", bufs=1))
        psB = ctx.enter_context(tc.tile_pool(name="psB", bufs=4, space="PSUM"))
        b4pool = ctx.enter_context(tc.tile_pool(name="b4p", bufs=3))
        psA_cm = tc.tile_pool(name="psA", bufs=4, space="PSUM")
        psA = psA_cm.__enter__()

        # ---- constant / input tiles
        pk1 = cpool.tile([128, PK1_W], BF16, tag="pk1")
        pk2 = cpool.tile([128, PK2_W], BF16, tag="pk2")
        packb = cpool.tile([128, 56], F32, tag="packb")
        xt = cpool.tile([120, 32 * 128], BF16, tag="xt")
        mrow = cpool.tile([1, XR * W], BF16, tag="mrow")
        sidx = cpool.tile([128, 402], I16, tag="sidx")

        packa = pk1[0:128, 0:PA_W]
        xb0 = pk1[0:128, PA_W : PA_W + 512]
        repl = pk2[0:128, 1024:1536]
        xbs = [xb0, pk2[0:128, 0:512], pk2[0:128, 512:1024]]

        w_cv1 = packa[0:128, PA_CV1 : PA_CV1 + 128]
        ident = packa[0:128, PA_ID : PA_ID + 128]
        ones128 = packa[0:1, PA_ONES : PA_ONES + 128]
        w_pxA = packa[0:32, PA_PXA : PA_PXA + 64]
        w_pxB = packa[0:32, PA_PXB : PA_PXB + 64]
        w_ecv1 = packa[0:65, PA_ECV : PA_ECV + 128]
        w_epxA = packa[0:51, PA_EPA : PA_EPA + 100]
        w_epxB = packa[0:50, PA_EPB : PA_EPB + 100]
        b_epx = packa[0:1, PA_EPBIAS : PA_EPBIAS + 100]
        w_edwb = packa[0:128, PA_EDW : PA_EDW + 26]
        w_dwp = packb[0:128, 0:25]
        b_dwp = packb[0:128, 25:26]
        w_edwp = packb[0:128, 26:51]
        b_edwp = packb[0:128, 51:52]
        b_cv1 = packb[0:128, 52:53]
        b_px = packb[0:64, 53:54]
        b_ecv1 = packb[0:128, 54:55]
        iota2 = sidx[0:128, 400:402]

        # ---- PE warm-up: memset scratch, then 8 junk matmuls so the HAM
        # clock gate reaches K=8/8 before the first real matmul (~3.4us of
        # sustained PE activity required; this overlaps the input DMAs)
        scr = work.tile([128, 512], BF16, tag="scr")
        scrf = work.tile([16, 4], F32, tag="scrf")
        psW = psA.tile([128, 512], F32, tag="convps")
        nc.vector.memset(scr[:], 0.0)
        nc.vector.memset(scrf[:], 0.0)
        for i in range(10):
            nc.tensor.matmul(
                psW[:], scr[:, 0:128], scr[:], start=True, stop=True
            )
        # preload the SiLU activation table off the critical path
        warm = work.tile([16, 16], BF16, tag="warm")
        nc.scalar.activation(warm[0:16, 0:2], scrf[0:16, 0:2], AF.Silu)

        # ---- queue the DMAs (per-engine FIFO order = priority order)
        nc.sync.dma_start(pk1[:], pk1_d)
        nc.sync.dma_start(pk2[:], pk2_d)
        nc.scalar.dma_start(mrow[:], mrow_d)
        nc.scalar.dma_start(packb[:], packb_d)
        nc.scalar.dma_start(sidx[:], sidx_d)
        for chk in range(2):
            nc.sync.dma_start(
                xt[:, 2048 * chk : 2048 * (chk + 1)],
                xt_d[:, 2048 * chk : 2048 * (chk + 1)],
            )

        psH = psB.tile([128, 512], F32, tag="small", name="psHeat")

        def heat(src_ap, kparts=128):
            nc.tensor.matmul(
                psH[0:32, 0:64], scr[0:kparts, 0:32], src_ap,
                start=True, stop=True,
            )

        # ---- persistent working tensors
        x1rep = work.tile([128, XR * W], BF16)     # cv1 out, 4x M-replicated
        x1p = work.tile([128, 9 * XW + 8], BF16)   # packed x1 (68-pitch)
        x2p = work.tile([128, 5 * XW], BF16)       # comp dw out slabs
        x2call = work.tile([32, 20 * W], BF16)     # dw out, rebased to 0:32
        e_in = work.tile([65, 20 * W], BF16)       # px out + mask row
        e1rep = work.tile([128, 20 * W], BF16)     # enc cv1 out (0:50, 64:114)
        e1p = work.tile([128, 12 * XW + 8], BF16)  # packed enc x1 (68-pitch)
        e2p = work.tile([128, 8 * XW], BF16)       # enc dw out slabs
        e2c64a = work.tile([50, 8 * W], BF16)      # enc dw out, 64-pitch
        e2c64b = work.tile([50, 8 * W], BF16)
        e2c64 = [e2c64a, e2c64b]
        ET = work.tile([128, 800], F32)            # enc px out (transposed)
        expv = work.tile([128, 800], F32)          # exp, [t][s][k]
        S = work.tile([128, 32], F32)              # [t][s]
        R = work.tile([128, 32], F32)
        wcat = work.tile([128, 800], BF16)         # [t][s][k]
        dall = work.tile([128, 3200], BF16)
        b4t = work.tile([128, 8 * 480], BF16)
        ostage = work.tile([128, 4096], BF16)      # [t, r, col] staging

        emask_t = e_in[64:65, :]
        nc.scalar.dma_start(emask_t, emask_d)

        x1p3 = x1p[:, 0 : 9 * XW].rearrange("p (r c) -> p r c", c=XW)
        x2p3 = x2p[:].rearrange("p (r c) -> p r c", c=XW)
        e1p3 = e1p[:, 0 : 12 * XW].rearrange("p (r c) -> p r c", c=XW)
        e2p3 = e2p[:].rearrange("p (r c) -> p r c", c=XW)

        # zero the slab tiles once (pad columns + unused partition rows)
        nc.vector.memset(x1p[:], 0.0)
        nc.vector.memset(e1p[:], 0.0)

        # ---- diag tiles for the PE dw lanes, built during the DMA window:
        # comp on DVE (tensor_scalar on ident), enc on gpsimd (local_scatter
        # with an iota index -- zero DVE cost, gpsimd is idle here)
        cdgs = []
        for i, t in enumerate(CD_PE):
            dgc = work.tile([128, 128], BF16, tag=f"dgc{i}")
            nc.vector.tensor_scalar(
                dgc[:], ident, w_dwp[:, t : t + 1], None, OP.mult
            )
            cdgs.append(dgc)
        edgs = []
        for i, t in enumerate(ED_PE):
            dge = work.tile([128, 128], BF16, tag=f"dge{i}")
            nc.gpsimd.local_scatter(
                dge[:], w_edwb[0:128, t : t + 2], iota2,
                channels=128, num_elems=128, num_idxs=2,
            )
            edgs.append(dge)

        # ---- comp cv1: 1x1 conv 128->32, M-replicated 4x (+ SiLU + mask)
        # stationary-weight-major order so the PE reuses each ldw 3x
        for ch in range(3):
            ps = psA.tile([128, 512], F32, tag="convps", name=f"psCV{ch}")
            nc.tensor.matmul(ps[:], w_cv1, xbs[ch], start=True, stop=False)
            nc.tensor.matmul(
                ps[:], ones128, mrow[:, 512 * ch : 512 * (ch + 1)],
                start=False, stop=True,
            )
            nc.scalar.activation(
                x1rep[:, 512 * ch : 512 * (ch + 1)], ps[:],
                AF.Silu, bias=b_cv1,
            )

        heat(x1rep[:, 0:64])

        # ---- pack x1 slabs: group g covers x1 rows 5g..5g+9 (engine copies)
        for g in range(4):
            src = x1rep[32 * g : 32 * g + 32, 5 * g * W : (5 * g + 9) * W] \
                .rearrange("p (r c) -> p r c", c=W)
            dst = x1p3[32 * g : 32 * g + 32, 0:9, 2 : 2 + W]
            if g % 2 == 0:
                nc.vector.tensor_copy(dst, src)
            else:
                nc.gpsimd.tensor_copy(dst, src)

        heat(x1p[:, 0:64])

        # ---- comp dw3/dw5 (unified 5x5 taps), split across engines
        FS = 5 * XW                    # 340
        acc_v0 = work.tile([128, FS], BF16)
        acc_v1 = work.tile([128, FS], BF16)
        acc_p = work.tile([128, FS], BF16)
        tmp0 = work.tile([128, FS], BF16)
        tmp1 = work.tile([128, FS], BF16)
        tmp2 = work.tile([128, FS], BF16)
        tmp3 = work.tile([128, FS], BF16)

        def dw_taps_dve(taps, accs, src, wcol, fs):
            for i, t in enumerate(taps):
                ty, tx = divmod(t, 5)
                sv = src[:, ty * XW + tx : ty * XW + tx + fs]
                av = accs[i % 2]
                if i < 2:
                    nc.vector.tensor_scalar(av, sv, wcol[:, t : t + 1], None, OP.mult)
                else:
                    nc.vector.scalar_tensor_tensor(
                        av, sv, wcol[:, t : t + 1], av, OP.mult, OP.add
                    )

        def dw_taps_pe(taps, psds, dgs, src, fs):
            # one diag tile per tap (PE may dedupe weight loads by address:
            # rewriting a ping-pong tile mid-group used stale weights on HW);
            # accumulation groups run sequentially, never interleaved
            nsp = len(psds)
            h = fs // nsp
            for j, psd in enumerate(psds):
                o0 = j * h
                n = h if j < nsp - 1 else fs - j * h
                for i, t in enumerate(taps):
                    ty, tx = divmod(t, 5)
                    o = ty * XW + tx + o0
                    nc.tensor.matmul(
                        psd[:, 0:n], dgs[i][:], src[:, o : o + n],
                        start=(i == 0), stop=(i == len(taps) - 1),
                    )

        def dw_taps_act_pool(taps, accp, tmps, src, wcol, fs):
            # Act scales the taps; DVE folds them in with cheap 2x-mode adds
            for i, t in enumerate(taps):
                ty, tx = divmod(t, 5)
                sv = src[:, ty * XW + tx : ty * XW + tx + fs]
                tmp = tmps[i % len(tmps)]
                nc.scalar.activation(tmp, sv, AF.Copy, scale=wcol[:, t : t + 1])
                if i == 1:
                    nc.vector.tensor_tensor(accp, tmps[0], tmps[1], OP.add)
                elif i >= 2:
                    nc.vector.tensor_tensor(accp, accp, tmp, OP.add)

        ctmps = [tmp0[:], tmp1[:], tmp2[:], tmp3[:]]
        psdc = psA.tile([128, FS], F32, tag="convps")
        dw_taps_dve(CD_DVE, [acc_v0[:], acc_v1[:]], x1p, w_dwp, FS)
        dw_taps_act_pool(CD_ACT, acc_p[:], ctmps, x1p, w_dwp, FS)
        dw_taps_pe(CD_PE, [psdc], cdgs, x1p, FS)
        heat(tmp1[:, 0:64])
        nc.vector.tensor_add(acc_v0[:], acc_v0[:], acc_v1[:])
        nc.vector.tensor_add(acc_v0[:], acc_v0[:], acc_p[:])
        nc.vector.tensor_add(acc_v0[:], acc_v0[:], psdc[:])
        nc.scalar.activation(x2p[:], acc_v0[:], AF.Silu, bias=b_dwp)
        heat(x2p[:, 0:64])

        # rebase the dw output slabs to partitions 0:32, 64-pitch contiguous
        for g in range(4):
            dst = x2call[0:32, 5 * g * W : (5 * g + 5) * W] \
                .rearrange("p (r c) -> p r c", c=W)
            src = x2p3[32 * g : 32 * g + 32, 0:5, 0:W]
            if g % 2 == 0:
                nc.vector.tensor_copy(dst, src)
            else:
                nc.scalar.copy(dst, src)

        # ---- comp px: 1x1 conv 64->64, K-split, stationary-major order
        pxg = ((0, 7), (7, 7), (14, 6))
        psPX = [psA.tile([64, nr * W], F32, tag="convps", name=f"psPX{i}")
                for i, (_, nr) in enumerate(pxg)]
        for i, (r0, nr) in enumerate(pxg):
            nc.tensor.matmul(
                psPX[i][:], w_pxA,
                x1rep[0:32, (r0 + 2) * W : (r0 + 2 + nr) * W],
                start=True, stop=False,
            )
        for i, (r0, nr) in enumerate(pxg):
            nc.tensor.matmul(
                psPX[i][:], w_pxB,
                x2call[0:32, r0 * W : (r0 + nr) * W],
                start=False, stop=True,
            )
        for i, (r0, nr) in enumerate(pxg):
            nc.scalar.activation(
                e_in[0:64, r0 * W : (r0 + nr) * W], psPX[i][:],
                AF.Silu, bias=b_px,
            )
            heat(e_in[0:64, r0 * W : r0 * W + 64], kparts=64)

        heat(e_in[0:64, 0:64], kparts=64)

        # ---- enc cv1: 1x1 conv 64->50, M-replicated 2x (mask rides K=65)
        ecg = ((0, 8), (8, 8), (16, 4))
        psEC = [psA.tile([128, 512], F32, tag="convps", name=f"psEC{i}")
                for i in range(len(ecg))]
        for i, (r0, nr) in enumerate(ecg):
            nc.tensor.matmul(
                psEC[i][:, : nr * W], w_ecv1,
                e_in[0:65, r0 * W : (r0 + nr) * W],
                start=True, stop=True,
            )
        for i, (r0, nr) in enumerate(ecg):
            nc.scalar.activation(
                e1rep[:, r0 * W : (r0 + nr) * W], psEC[i][:, : nr * W],
                AF.Silu, bias=b_ecv1,
            )

        heat(e1rep[:, 0:64])

        # ---- pack enc slabs: group g covers e-rows 8g..8g+12
        for g in range(2):
            p0 = 64 * g
            src = e1rep[p0 : p0 + 50, 8 * g * W : (8 * g + 12) * W] \
                .rearrange("p (r c) -> p r c", c=W)
            dst = e1p3[p0 : p0 + 50, 0:12, 2 : 2 + W]
            if g == 0:
                nc.vector.tensor_copy(dst, src)
            else:
                nc.gpsimd.tensor_copy(dst, src)

        heat(e1p[:, 0:64])

        # ---- enc dw3/dw5, same three-engine split
        FS2 = 8 * XW                   # 544
        eacc_v0 = work.tile([128, FS2], BF16)
        eacc_v1 = work.tile([128, FS2], BF16)
        eacc_p = work.tile([128, FS2], BF16)
        etmp0 = work.tile([128, FS2], BF16)
        etmp1 = work.tile([128, FS2], BF16)
        etmp2 = work.tile([128, FS2], BF16)
        etmp3 = work.tile([128, FS2], BF16)
        etmps = [etmp0[:], etmp1[:], etmp2[:], etmp3[:]]
        psde0 = psA.tile([128, FS2 // 2], F32, tag="convps")
        psde1 = psA.tile([128, FS2 // 2], F32, tag="convps")
        dw_taps_dve(ED_DVE, [eacc_v0[:], eacc_v1[:]], e1p, w_edwp, FS2)
        dw_taps_act_pool(ED_ACT, eacc_p[:], etmps, e1p, w_edwp, FS2)
        dw_taps_pe(ED_PE, [psde0, psde1], edgs, e1p, FS2)
        heat(etmp1[:, 0:64])
        nc.vector.tensor_add(eacc_v0[:], eacc_v0[:], eacc_v1[:])
        nc.vector.tensor_add(eacc_v0[:], eacc_v0[:], eacc_p[:])
        nc.vector.tensor_add(
            eacc_v0[:, 0 : FS2 // 2], eacc_v0[:, 0 : FS2 // 2], psde0[:]
        )
        nc.vector.tensor_add(
            eacc_v0[:, FS2 // 2 : FS2], eacc_v0[:, FS2 // 2 : FS2], psde1[:]
        )
        nc.scalar.activation(e2p[:], eacc_v0[:], AF.Silu, bias=b_edwp)
        heat(e2p[:, 0:64])

        # repack both groups to base 0, 64-pitch contiguous
        for g in range(2):
            dst = e2c64[g][:].rearrange("p (r c) -> p r c", c=W)
            src = e2p3[64 * g : 64 * g + 50, 0:8, 0:W]
            if g == 0:
                nc.vector.tensor_copy(dst, src)
            else:
                nc.scalar.copy(dst, src)

        # ---- enc px (transposed output; K-split e1 + e2 + bias row),
        # emitted in two halves so softmax/backend stage A starts while the
        # t4-7 SiLU acts still run (costs two extra act-table switches on
        # Act, wins ~4us of backend start time)
        def epx_half(ts):
            for t in ts:
                g, lr = divmod(t, 4)
                ps = psA.tile([128, 100], F32, tag="convps", name=f"psET{t}")
                nc.tensor.matmul(
                    ps[:], e1rep[0:51, (2 * t + 2) * W : (2 * t + 4) * W],
                    w_epxA, start=True, stop=False,
                )
                nc.tensor.matmul(
                    ps[:], e2c64[g][:, 2 * lr * W : (2 * lr + 2) * W],
                    w_epxB, start=False, stop=True,
                )
                nc.scalar.activation(
                    ET[:, 100 * t : 100 * t + 100], ps[:], AF.Silu
                )

        # ---- softmax over 25 taps (no max-subtraction)
        # ET channel e within t is (k, s) raster: e = 4k + s
        expw = work.tile([128, 800], BF16)         # exp, [t][s][k] bf16
        ET_tsk = ET[:].rearrange("p (t k s) -> p t s k", t=8, k=25, s=4)
        exp4 = expw[:].rearrange("p (t s k) -> p t s k", t=8, s=4)
        S3 = S[:].rearrange("p (t s) -> p t s", s=4)
        R3 = R[:].rearrange("p (t s) -> p t s", s=4)
        wcat4 = wcat[:].rearrange("p (t s k) -> p t s k", t=8, s=4)

        def fence(lo, hi):
            # reads one element of each ET act range [lo, hi) and writes
            # inside that exp's output, so Tile cannot hoist the Exp (and
            # its act-table load) above those SiLU acts
            fsrc = ET[:, 100 * lo : 100 * hi] \
                .rearrange("p (t e) -> p t e", e=100)[:, :, 99]
            nc.scalar.copy(expw[:, 400 - (hi - lo) : 400], fsrc)

        def softmax_stage(lo, hi):
            nc.scalar.activation(exp4[:, lo:hi], ET_tsk[:, lo:hi], AF.Exp)
            nc.vector.tensor_reduce(S3[:, lo:hi], exp4[:, lo:hi], AX.X, OP.add)
            nc.vector.reciprocal(R[:, 4 * lo : 4 * hi], S[:, 4 * lo : 4 * hi])
            R4 = (
                R3[:, lo:hi]
                .unsqueeze(-1)
                .to_broadcast((128, hi - lo, 4, 25))
            )
            nc.vector.tensor_tensor(
                wcat4[:, lo:hi], exp4[:, lo:hi], R4, OP.mult
            )

        def backend_stage(t0, dve_only):
            # repl matmuls jb-major so each repl lhsT is reused 4x; all 4
            # jb's of one t land in a single bf16 PSUM bank -> one copy +
            # one scatter per t
            pss = [psB.tile([128, 512], F32, tag="small", name=f"pss{t0}_{i}")
                   for i in range(4)]
            for jb in range(4):
                for dt in range(4):
                    t = t0 + dt
                    nc.tensor.matmul(
                        pss[dt][:, 100 * jb : 100 * jb + 100],
                        repl[:, 128 * jb : 128 * jb + 128],
                        wcat[:, 100 * t : 100 * t + 100],
                        start=True, stop=True,
                    )
            for dt in range(4):
                t = t0 + dt
                dst = dall[:, 400 * t : 400 * t + 400]
                if dve_only:
                    nc.vector.tensor_copy(dst, pss[dt][0:128, 0:400])
                else:
                    nc.scalar.copy(dst, pss[dt][0:128, 0:400])
                nc.gpsimd.local_scatter(
                    b4t[:, 480 * t : 480 * t + 480],
                    dall[:, 400 * t : 400 * t + 400],
                    sidx[0:128, 0:400],
                    channels=128, num_elems=480, num_idxs=400,
                )

        epx_half(range(0, 4))
        epx_half(range(4, 8))
        fence(0, 8)
        softmax_stage(0, 4)
        backend_stage(0, dve_only=False)
        softmax_stage(4, 8)
        backend_stage(4, dve_only=True)

        psA_cm.__exit__(None, None, None)
        psC = ctx.enter_context(tc.tile_pool(name="psC", bufs=2, space="PSUM"))
        psO = ctx.enter_context(tc.tile_pool(name="psO", bufs=2, space="PSUM"))

        for t in range(8):
            # 4 transposes land in one [120, 512] bf16 PSUM bank
            psb4 = psC.tile([120, 1024], BF16, tag="b4t")
            for jb in range(4):
                nc.tensor.transpose(
                    psb4[:, 128 * jb : 128 * jb + 128],
                    b4t[:, 480 * t + 120 * jb : 480 * t + 120 * jb + 120],
                    ident,
                )
            b4 = b4pool.tile([120, 512], BF16, tag="b4")
            if t % 2 == 0:
                nc.scalar.copy(b4[:], psb4[0:120, 0:512])
            else:
                nc.vector.tensor_copy(b4[:], psb4[0:120, 0:512])

            po = psO.tile([128, 512], F32, tag="out")
            for jb in range(4):
                B = 4 * t + jb
                nc.tensor.matmul(
                    po[:, 128 * jb : 128 * jb + 128],
                    xt[:, 128 * B : 128 * B + 128],
                    b4[:, 128 * jb : 128 * jb + 128],
                    start=True, stop=True,
                )
            # one staging copy per t: po columns are (jb, r, j) raster,
            # ostage wants (r, jb, j)
            src = po[:].rearrange("c (b r j) -> c b r j", b=4, j=32)
            dst = ostage[:, 512 * t : 512 * (t + 1)] \
                .rearrange("c (r b j) -> c b r j", b=4, j=32)
            if t % 2 == 0:
                nc.vector.tensor_copy(dst, src)
            else:
                nc.scalar.copy(dst, src)
            if t in (1, 3, 5):
                eng = nc.sync if t % 4 == 1 else nc.scalar
                eng.dma_start(
                    out_d[:, 512 * (t - 1) : 512 * (t + 1)],
                    ostage[:, 512 * (t - 1) : 512 * (t + 1)],
                )
            elif t >= 6:
                eng = nc.sync if t == 6 else nc.scalar
                eng.dma_start(
                    out_d[:, 512 * t : 512 * (t + 1)],
                    ostage[:, 512 * t : 512 * (t + 1)],
                )

    nc.compile()
    return nc


_NC_CACHE = None


def _get_nc():
    global _NC_CACHE
    if _NC_CACHE is None:
        _NC_CACHE = build_kernel()
    return _NC_CACHE


def _make_in_maps(inputs):
    X = np.asarray(inputs["X"], np.float32)
    consts = _host_consts(
        {k: np.asarray(v, np.float32) for k, v in inputs.items() if k != "X"}
    )
    in_maps = []
    for core in range(NCORES):
        xs, mrow, emask, xt = _host_shard(X, core)
        pk1 = np.zeros((128, PK1_W), nbf)
        pk1[:, 0:PA_W] = consts["_pa"]
        pk1[:, PA_W : PA_W + 512] = xs[:, 0:512]
        pk2 = np.zeros((128, PK2_W), nbf)
        pk2[:, 0:1024] = xs[:, 512:1536]
        pk2[:, 1024:1536] = consts["_repl"]
        m = {
            "packb": consts["packb"],
            "sidx": consts["sidx"],
            "pk1": pk1,
            "pk2": pk2,
            "mrow": mrow,
            "emask": emask,
            "xt": xt,
        }
        in_maps.append(m)
    return in_maps


def kernel(**inputs) -> np.ndarray:
    in_maps = _make_in_maps(inputs)
    nc = _get_nc()
    res = run_bass_kernel_spmd(nc, in_maps, core_ids=list(range(NCORES)))
    out = np.zeros((2, C, 128, 128), np.float32)
    for core in range(NCORES):
        b, ri = divmod(core, 4)
        # out dram layout: [c, (t, r, col)]
        blk = res.results[core]["out"].astype(np.float32).reshape(C, 32, 128)
        out[b, :, 32 * ri : 32 * ri + 32, :] = blk
    return out


if __name__ == "__main__":
    print("smoke build only")
    build_kernel()
    print("build ok")


# revision 20
# speedup vs baseline: 1.1769x; 1.1769x over previous
"""CARAFE + MSGConv Trainium2 kernel (8 NeuronCores, spatial x batch sharding).

out[c, i, j] = sum_{p,q} W[5p+q, i, j] * Xpad[c, i//2 + p - 2, j//2 + q - 2]
 (CARAFE taps live at source resolution; identical for both subpixel parities).

Per core: one batch element (core//4) and a 16-source-row block (core%4).
The 25-tap reassembly runs on the TensorEngine as one K=120 matmul per
(row-pair, column-quarter) block:
  out[c, n] = sum_{(u,v)} X6T[(u,v), c] * B4[(u,v), n]
where B4 is a banded matrix of softmaxed W values built at runtime with
gpsimd local_scatter (per-partition index scatter) + a PE transpose; the
X side (X6T) is static data and comes pre-transposed from the host.

v3 restructure vs v2:
- one big input DMA (x | packa | repl) + xt + 4 small ones; ident moved
  into the early pack so the diag builds overlap the DMA window
- PE warm-up burst at t=0 (8 junk matmuls on a memset scratch) so HAM
  un-throttles to K=8/8 before the first real matmul
- stationary-weight reuse ordering: cv1/px run all matmuls sharing one
  lhsT back-to-back (banked PSUM accumulation groups interleaved)
- diag tiles: comp built by DVE, enc built by gpsimd local_scatter with
  an iota index (both overlap the head phase)
- backend coarsened: 4 repl matmuls land in one [128,400] PSUM bank ->
  one cast per t; 4 transposes land in one [120,512] bf16 bank -> one
  b4 copy per t; finals land in one [128,512] bank -> one stg copy per
  t into a persistent [128,4096] staging tile; 4 contiguous out DMAs
- output DRAM layout is t-major contiguous (no strided descriptors)
"""

import sys

sys.path.insert(0, "/opt/trn_rl_repo")

from contextlib import ExitStack

import ml_dtypes
import numpy as np

import concourse.bass as bass
import concourse.tile as tile
from concourse import bacc, library_config, mybir
from concourse.bass_utils import run_bass_kernel_spmd

BF16 = mybir.dt.bfloat16
F32 = mybir.dt.float32
I16 = mybir.dt.int16
AF = mybir.ActivationFunctionType
OP = mybir.AluOpType
AX = mybir.AxisListType
nbf = ml_dtypes.bfloat16

C = 128
H = W = 64
NCORES = 8
XR = 24          # X shard rows (16 + 4 halo each side)
XW = 68          # padded width for dw slabs only
NEG = -30.0      # additive pre-activation mask; SiLU(-30) ~= -2.8e-12

# packa column layout (bf16), all DMA'd in one shot with x
PA_CV1 = 0       # w_cv1_rep   [128, 128]
PA_ID = 128      # ident       [128, 128]
PA_ONES = 256    # ones row    [1, 128]
PA_PXA = 384     # w_pxA       [32, 64]
PA_PXB = 448     # w_pxB       [32, 64]
PA_ECV = 512     # w_ecv1_rep  [65, 128] (M groups at 0:50 and 64:114)
PA_EPA = 640     # w_epxA      [51, 100]
PA_EPB = 740     # w_epxB      [50, 100]
PA_EPBIAS = 840  # b_epx_row   [1, 100]
PA_EDW = 940     # w_edwp bf16 [128, 25] (for gpsimd diag builds)
PA_W = 966

PK1_W = PA_W + 512       # pk1: [packa | x[:, 0:512]]
PK2_W = 1536             # pk2: [x[:, 512:1536] | repl]

# comp dw tap split: DVE / Act / PE
CD_DVE = list(range(0, 7))
CD_ACT = list(range(7, 12))
CD_PE = list(range(12, 25))
# enc dw tap split
ED_DVE = list(range(0, 7))
ED_ACT = list(range(7, 11))
ED_PE = list(range(11, 25))


# ======================================================================
# host-side parameter prep
# ======================================================================

def _fold_1x1(w, s):
    return (w[:, :, 0, 0] * s[:, None]).T.copy()


def _dw_taps(w, s, k):
    ch = w.shape[0]
    out = np.zeros((ch, 25), np.float32)
    off = (5 - k) // 2
    for ty in range(k):
        for tx in range(k):
            out[:, 5 * (ty + off) + (tx + off)] = w[:, 0, ty, tx] * s
    return out


def _host_consts(inputs):
    d = {}
    w_cv1 = _fold_1x1(inputs["comp_cv1_w"], inputs["comp_cv1_s"])   # [128, 32]
    b_cv1 = inputs["comp_cv1_b"]                                    # [32]
    w3 = _dw_taps(inputs["comp_dw3_w"], inputs["comp_dw3_s"], 3)
    w5 = _dw_taps(inputs["comp_dw5_w"], inputs["comp_dw5_s"], 5)
    w_dwp = np.tile(np.concatenate([w3, w5], 0), (4, 1))            # [128, 25]
    b_dwp = np.tile(
        np.concatenate([inputs["comp_dw3_b"], inputs["comp_dw5_b"]]), 4
    ).reshape(128, 1)
    w_px = _fold_1x1(inputs["comp_px_w"], inputs["comp_px_s"])      # [64, 64]
    b_px = inputs["comp_px_b"].reshape(64, 1)
    we = _fold_1x1(inputs["enc_cv1_w"], inputs["enc_cv1_s"])        # [64, 50]
    w_ecv1 = np.concatenate([we, np.ones((1, 50), np.float32)], 0)  # [65, 50]
    b_ecv1 = inputs["enc_cv1_b"]                                    # [50]
    e3 = _dw_taps(inputs["enc_dw3_w"], inputs["enc_dw3_s"], 3)
    e5 = _dw_taps(inputs["enc_dw5_w"], inputs["enc_dw5_s"], 5)
    w_edw50 = np.concatenate([e3, e5], 0)                           # [50, 25]
    b_edw50 = np.concatenate(
        [inputs["enc_dw3_b"], inputs["enc_dw5_b"]]
    ).reshape(50, 1)
    wpx_e = _fold_1x1(inputs["enc_px_w"], inputs["enc_px_s"])       # [100, 100]
    b_epx = inputs["enc_px_b"].reshape(1, 100)

    w_edwp = np.zeros((128, 25), np.float32)
    w_edwp[0:50] = w_edw50
    w_edwp[64:114] = w_edw50

    pa = np.zeros((128, PA_W), np.float32)
    pa[:, PA_CV1 : PA_CV1 + 128] = np.tile(w_cv1, (1, 4))
    pa[:, PA_ID : PA_ID + 128] = np.eye(128)
    pa[0:1, PA_ONES : PA_ONES + 128] = 1.0
    pa[0:32, PA_PXA : PA_PXA + 64] = w_px[0:32]
    pa[0:32, PA_PXB : PA_PXB + 64] = w_px[32:64]
    pa[0:65, PA_ECV : PA_ECV + 50] = w_ecv1
    pa[0:65, PA_ECV + 64 : PA_ECV + 114] = w_ecv1
    pa[0:50, PA_EPA : PA_EPA + 100] = wpx_e[0:50]
    pa[50:51, PA_EPA : PA_EPA + 100] = b_epx
    pa[0:50, PA_EPB : PA_EPB + 100] = wpx_e[50:100]
    pa[0:1, PA_EPBIAS : PA_EPBIAS + 100] = b_epx
    pa[:, PA_EDW : PA_EDW + 25] = w_edwp
    d["_pa"] = pa.astype(nbf)

    pb = np.zeros((128, 56), np.float32)
    pb[:, 0:25] = w_dwp
    pb[:, 25:26] = b_dwp
    pb[0:50, 26:51] = w_edw50
    pb[64:114, 26:51] = w_edw50
    pb[0:50, 51:52] = b_edw50
    pb[64:114, 51:52] = b_edw50
    pb[:, 52:53] = np.tile(b_cv1, 4).reshape(128, 1)
    pb[0:64, 53:54] = b_px
    pb[0:50, 54:55] = b_ecv1.reshape(50, 1)
    pb[64:114, 54:55] = b_ecv1.reshape(50, 1)
    # SiLU(1.2784645) = 1: the ecv1 act writes a ones row at partition 50,
    # which carries the enc-px bias via K=51 on the e1-part matmul
    pb[50:51, 54:55] = 1.2784645427610737
    d["packb"] = pb

    # repl [128, 4*128]: lhsT for the W row-replication matmul
    # n raster within a block: n = 32*(2*yl+dy) + (2*xl+dx)
    rp = np.zeros((128, 512), np.float32)
    for jb in range(4):
        for n in range(128):
            rho, j = divmod(n, 32)
            yl, xl = rho // 2, j // 2
            rp[64 * yl + 16 * jb + xl, 128 * jb + n] = 1.0
    d["_repl"] = rp.astype(nbf)

    # sidx [128, 400+2] int16; cols 0:400 = scatter map (4 blocks per t),
    # cols 400:402 = iota idx for the gpsimd diag builds.
    si = np.full((128, 402), -1, np.int16)
    for n in range(128):
        rho, j = divmod(n, 32)
        yl, dy = divmod(rho, 2)
        xl, dx = divmod(j, 2)
        sn = 2 * dy + dx
        for jb in range(4):
            for cp in range(100):
                sc, k = divmod(cp, 25)
                if sc != sn:
                    continue
                p, q = divmod(k, 5)
                if not (0 <= 16 * jb + xl + q - 2 < 64):
                    continue
                si[n, 100 * jb + cp] = 120 * jb + 20 * (yl + p) + (xl + q)
        si[n, 400] = n
    d["sidx"] = si
    return d


def _host_shard(X, core):
    b, ri = divmod(core, 4)
    r0 = 16 * ri - 4
    xs = np.zeros((C, XR, W), np.float32)
    lo, hi = max(0, r0), min(H, r0 + XR)
    xs[:, lo - r0 : hi - r0, :] = X[b, :, lo:hi, :]
    mrow = np.zeros((1, XR, W), np.float32)
    for r in range(XR):
        if not (0 <= r0 + r < H):
            mrow[0, r, :] = NEG
    emask = np.zeros((1, 20, W), np.float32)
    for r in range(20):
        if not (0 <= (16 * ri - 2) + r < H):
            emask[0, r, :] = NEG
    xsb = xs.astype(nbf)
    # pre-transposed X slabs, one [120, 128] per block (column-padded)
    xsp = np.zeros((C, XR, XW), nbf)
    xsp[:, :, 2 : 2 + W] = xsb
    xt = np.zeros((120, 32 * 128), nbf)
    for B in range(32):
        t, jb = divmod(B, 4)
        slab = xsp[:, 2 * t + 2 : 2 * t + 8, 16 * jb : 16 * jb + 20]
        xt[:, 128 * B : 128 * B + 128] = slab.reshape(C, 120).T
    return (
        xsb.reshape(C, XR * W),
        mrow.reshape(1, XR * W).astype(nbf),
        emask.reshape(1, 20 * W).astype(nbf),
        xt,
    )


# ======================================================================
# device kernel
# ======================================================================

def build_kernel():
    nc = bacc.Bacc(
        "TRN2",
        target_bir_lowering=False,
        debug=False,
        enable_asserts=False,
        num_devices=NCORES,
    )

    def din(name, shape, dt):
        return nc.dram_tensor(name, list(shape), dt, kind="ExternalInput").ap()

    pk1_d = din("pk1", (128, PK1_W), BF16)
    pk2_d = din("pk2", (128, PK2_W), BF16)
    xt_d = din("xt", (120, 32 * 128), BF16)
    mrow_d = din("mrow", (1, XR * W), BF16)
    emask_d = din("emask", (1, 20 * W), BF16)
    packb_d = din("packb", (128, 56), F32)
    sidx_d = din("sidx", (128, 402), I16)
    # out layout: [c, (t, r, col)] -- t-major contiguous
    out_d = nc.dram_tensor("out", [128, 32 * 128], BF16, kind="ExternalOutput").ap()

    with tile.TileContext(nc) as tc, ExitStack() as ctx:
        cpool = ctx.enter_context(tc.tile_pool(name="consts", bufs=1))
        work = ctx.enter_context(tc.tile_pool(name="work", bufs=1))
        psB = ctx.enter_context(tc.tile_pool(name="psB", bufs=4, space="PSUM"))
        b4pool = ctx.enter_context(tc.tile_pool(name="b4p", bufs=3))
        psA_cm = tc.tile_pool(name="psA", bufs=4, space="PSUM")
        psA = psA_cm.__enter__()

        # ---- constant / input tiles
        pk1 = cpool.tile([128, PK1_W], BF16, tag="pk1")
        pk2 = cpool.tile([128, PK2_W], BF16, tag="pk2")
        packb = cpool.tile([128, 56], F32, tag="packb")
        xt = cpool.tile([120, 32 * 128], BF16, tag="xt")
        mrow = cpool.tile([1, XR * W], BF16, tag="mrow")
        sidx = cpool.tile([128, 402], I16, tag="sidx")

        packa = pk1[0:128, 0:PA_W]
        xb0 = pk1[0:128, PA_W : PA_W + 512]
        repl = pk2[0:128, 1024:1536]
        xbs = [xb0, pk2[0:128, 0:512], pk2[0:128, 512:1024]]

        w_cv1 = packa[0:128, PA_CV1 : PA_CV1 + 128]
        ident = packa[0:128, PA_ID : PA_ID + 128]
        ones128 = packa[0:1, PA_ONES : PA_ONES + 128]
        w_pxA = packa[0:32, PA_PXA : PA_PXA + 64]
        w_pxB = packa[0:32, PA_PXB : PA_PXB + 64]
        w_ecv1 = packa[0:65, PA_ECV : PA_ECV + 128]
        w_epxA = packa[0:51, PA_EPA : PA_EPA + 100]
        w_epxB = packa[0:50, PA_EPB : PA_EPB + 100]
        b_epx = packa[0:1, PA_EPBIAS : PA_EPBIAS + 100]
        w_edwb = packa[0:128, PA_EDW : PA_EDW + 26]
        w_dwp = packb[0:128, 0:25]
        b_dwp = packb[0:128, 25:26]
        w_edwp = packb[0:128, 26:51]
        b_edwp = packb[0:128, 51:52]
        b_cv1 = packb[0:128, 52:53]
        b_px = packb[0:64, 53:54]
        b_ecv1 = packb[0:128, 54:55]
        iota2 = sidx[0:128, 400:402]

        # ---- PE warm-up: memset scratch, then 8 junk matmuls so the HAM
        # clock gate reaches K=8/8 before the first real matmul (~3.4us of
        # sustained PE activity required; this overlaps the input DMAs)
        scr = work.tile([128, 512], BF16, tag="scr")
        scrf = work.tile([16, 4], F32, tag="scrf")
        psW = psA.tile([128, 512], F32, tag="convps")
        nc.vector.memset(scr[:], 0.0)
        nc.vector.memset(scrf[:], 0.0)
        for i in range(10):
            nc.tensor.matmul(
                psW[:], scr[:, 0:128], scr[:], start=True, stop=True
            )
        # preload the SiLU activation table off the critical path
        warm = work.tile([16, 16], BF16, tag="warm")
        nc.scalar.activation(warm[0:16, 0:2], scrf[0:16, 0:2], AF.Silu)

        # ---- queue the DMAs (per-engine FIFO order = priority order)
        nc.sync.dma_start(pk1[:], pk1_d)
        nc.sync.dma_start(pk2[:], pk2_d)
        nc.scalar.dma_start(mrow[:], mrow_d)
        nc.scalar.dma_start(packb[:], packb_d)
        nc.scalar.dma_start(sidx[:], sidx_d)
        for chk in range(2):
            nc.sync.dma_start(
                xt[:, 2048 * chk : 2048 * (chk + 1)],
                xt_d[:, 2048 * chk : 2048 * (chk + 1)],
            )

        psH = psB.tile([128, 512], F32, tag="small", name="psHeat")

        def heat(src_ap, kparts=128):
            nc.tensor.matmul(
                psH[0:32, 0:64], scr[0:kparts, 0:32], src_ap,
                start=True, stop=True,
            )

        # ---- persistent working tensors
        x1rep = work.tile([128, XR * W], BF16)     # cv1 out, 4x M-replicated
        x1p = work.tile([128, 9 * XW + 8], BF16)   # packed x1 (68-pitch)
        x2p = work.tile([128, 5 * XW], BF16)       # comp dw out slabs
        x2call = work.tile([32, 20 * W], BF16)     # dw out, rebased to 0:32
        e_in = work.tile([65, 20 * W], BF16)       # px out + mask row
        e1rep = work.tile([128, 20 * W], BF16)     # enc cv1 out (0:50, 64:114)
        e1p = work.tile([128, 12 * XW + 8], BF16)  # packed enc x1 (68-pitch)
        e2p = work.tile([128, 8 * XW], BF16)       # enc dw out slabs
        e2c64a = work.tile([50, 8 * W], BF16)      # enc dw out, 64-pitch
        e2c64b = work.tile([50, 8 * W], BF16)
        e2c64 = [e2c64a, e2c64b]
        ET = work.tile([128, 800], F32)            # enc px out (transposed)
        expv = work.tile([128, 800], F32)          # exp, [t][s][k]
        S = work.tile([128, 32], F32)              # [t][s]
        R = work.tile([128, 32], F32)
        wcat = work.tile([128, 800], BF16)         # [t][s][k]
        dall = work.tile([128, 3200], BF16)
        b4t = work.tile([128, 8 * 480], BF16)
        ostage = work.tile([128, 4096], BF16)      # [t, r, col] staging

        emask_t = e_in[64:65, :]
        nc.scalar.dma_start(emask_t, emask_d)

        x1p3 = x1p[:, 0 : 9 * XW].rearrange("p (r c) -> p r c", c=XW)
        x2p3 = x2p[:].rearrange("p (r c) -> p r c", c=XW)
        e1p3 = e1p[:, 0 : 12 * XW].rearrange("p (r c) -> p r c", c=XW)
        e2p3 = e2p[:].rearrange("p (r c) -> p r c", c=XW)

        # zero the slab tiles once (pad columns + unused partition rows)
        nc.vector.memset(x1p[:], 0.0)
        nc.vector.memset(e1p[:], 0.0)

        # ---- diag tiles for the PE dw lanes, built during the DMA window:
        # comp on DVE (tensor_scalar on ident), enc on gpsimd (local_scatter
        # with an iota index -- zero DVE cost, gpsimd is idle here)
        cdgs = []
        for i, t in enumerate(CD_PE):
            dgc = work.tile([128, 128], BF16, tag=f"dgc{i}")
            nc.vector.tensor_scalar(
                dgc[:], ident, w_dwp[:, t : t + 1], None, OP.mult
            )
            cdgs.append(dgc)
        edgs = []
        for i, t in enumerate(ED_PE):
            dge = work.tile([128, 128], BF16, tag=f"dge{i}")
            nc.gpsimd.local_scatter(
                dge[:], w_edwb[0:128, t : t + 2], iota2,
                channels=128, num_elems=128, num_idxs=2,
            )
            edgs.append(dge)

        # ---- comp cv1: 1x1 conv 128->32, M-replicated 4x (+ SiLU + mask)
        # stationary-weight-major order so the PE reuses each ldw 3x
        for ch in range(3):
            ps = psA.tile([128, 512], F32, tag="convps", name=f"psCV{ch}")
            nc.tensor.matmul(ps[:], w_cv1, xbs[ch], start=True, stop=False)
            nc.tensor.matmul(
                ps[:], ones128, mrow[:, 512 * ch : 512 * (ch + 1)],
                start=False, stop=True,
            )
            nc.scalar.activation(
                x1rep[:, 512 * ch : 512 * (ch + 1)], ps[:],
                AF.Silu, bias=b_cv1,
            )

        heat(x1rep[:, 0:64])

        # ---- pack x1 slabs: group g covers x1 rows 5g..5g+9 (engine copies)
        for g in range(4):
            src = x1rep[32 * g : 32 * g + 32, 5 * g * W : (5 * g + 9) * W] \
                .rearrange("p (r c) -> p r c", c=W)
            dst = x1p3[32 * g : 32 * g + 32, 0:9, 2 : 2 + W]
            if g % 2 == 0:
                nc.vector.tensor_copy(dst, src)
            else:
                nc.scalar.copy(dst, src)

        heat(x1p[:, 0:64])

        # ---- comp dw3/dw5 (unified 5x5 taps), split across engines
        FS = 5 * XW                    # 340
        acc_v0 = work.tile([128, FS], BF16)
        acc_v1 = work.tile([128, FS], BF16)
        acc_p = work.tile([128, FS], BF16)
        tmp0 = work.tile([128, FS], BF16)
        tmp1 = work.tile([128, FS], BF16)
        tmp2 = work.tile([128, FS], BF16)
        tmp3 = work.tile([128, FS], BF16)

        def dw_taps_dve(taps, accs, src, wcol, fs):
            for i, t in enumerate(taps):
                ty, tx = divmod(t, 5)
                sv = src[:, ty * XW + tx : ty * XW + tx + fs]
                av = accs[i % 2]
                if i < 2:
                    nc.vector.tensor_scalar(av, sv, wcol[:, t : t + 1], None, OP.mult)
                else:
                    nc.vector.scalar_tensor_tensor(
                        av, sv, wcol[:, t : t + 1], av, OP.mult, OP.add
                    )

        def dw_taps_pe(taps, psds, dgs, src, fs):
            # one diag tile per tap (PE may dedupe weight loads by address:
            # rewriting a ping-pong tile mid-group used stale weights on HW);
            # accumulation groups run sequentially, never interleaved
            nsp = len(psds)
            h = fs // nsp
            for j, psd in enumerate(psds):
                o0 = j * h
                n = h if j < nsp - 1 else fs - j * h
                for i, t in enumerate(taps):
                    ty, tx = divmod(t, 5)
                    o = ty * XW + tx + o0
                    nc.tensor.matmul(
                        psd[:, 0:n], dgs[i][:], src[:, o : o + n],
                        start=(i == 0), stop=(i == len(taps) - 1),
                    )

        def dw_taps_act_pool(taps, accp, tmps, src, wcol, fs):
            # Act scales the taps; DVE folds them in with cheap 2x-mode adds
            for i, t in enumerate(taps):
                ty, tx = divmod(t, 5)
                sv = src[:, ty * XW + tx : ty * XW + tx + fs]
                tmp = tmps[i % len(tmps)]
                nc.scalar.activation(tmp, sv, AF.Copy, scale=wcol[:, t : t + 1])
                if i == 1:
                    nc.vector.tensor_tensor(accp, tmps[0], tmps[1], OP.add)
                elif i >= 2:
                    nc.vector.tensor_tensor(accp, accp, tmp, OP.add)

        ctmps = [tmp0[:], tmp1[:], tmp2[:], tmp3[:]]
        psdc = psA.tile([128, FS], F32, tag="convps")
        dw_taps_dve(CD_DVE, [acc_v0[:], acc_v1[:]], x1p, w_dwp, FS)
        dw_taps_act_pool(CD_ACT, acc_p[:], ctmps, x1p, w_dwp, FS)
        dw_taps_pe(CD_PE, [psdc], cdgs, x1p, FS)
        heat(tmp1[:, 0:64])
        nc.vector.tensor_add(acc_v0[:], acc_v0[:], acc_v1[:])
        nc.vector.tensor_add(acc_v0[:], acc_v0[:], acc_p[:])
        nc.vector.tensor_add(acc_v0[:], acc_v0[:], psdc[:])
        nc.scalar.activation(x2p[:], acc_v0[:], AF.Silu, bias=b_dwp)
        heat(x2p[:, 0:64])

        # rebase the dw output slabs to partitions 0:32, 64-pitch contiguous
        for g in range(4):
            dst = x2call[0:32, 5 * g * W : (5 * g + 5) * W] \
                .rearrange("p (r c) -> p r c", c=W)
            src = x2p3[32 * g : 32 * g + 32, 0:5, 0:W]
            if g % 2 == 0:
                nc.vector.tensor_copy(dst, src)
            else:
                nc.scalar.copy(dst, src)

        # ---- comp px: 1x1 conv 64->64, K-split, stationary-major order
        pxg = ((0, 7), (7, 7), (14, 6))
        psPX = [psA.tile([64, nr * W], F32, tag="convps", name=f"psPX{i}")
                for i, (_, nr) in enumerate(pxg)]
        for i, (r0, nr) in enumerate(pxg):
            nc.tensor.matmul(
                psPX[i][:], w_pxA,
                x1rep[0:32, (r0 + 2) * W : (r0 + 2 + nr) * W],
                start=True, stop=False,
            )
        for i, (r0, nr) in enumerate(pxg):
            nc.tensor.matmul(
                psPX[i][:], w_pxB,
                x2call[0:32, r0 * W : (r0 + nr) * W],
                start=False, stop=True,
            )
        for i, (r0, nr) in enumerate(pxg):
            nc.scalar.activation(
                e_in[0:64, r0 * W : (r0 + nr) * W], psPX[i][:],
                AF.Silu, bias=b_px,
            )
            heat(e_in[0:64, r0 * W : r0 * W + 64], kparts=64)

        heat(e_in[0:64, 0:64], kparts=64)

        # ---- enc cv1: 1x1 conv 64->50, M-replicated 2x (mask rides K=65)
        ecg = ((0, 8), (8, 8), (16, 4))
        psEC = [psA.tile([128, 512], F32, tag="convps", name=f"psEC{i}")
                for i in range(len(ecg))]
        for i, (r0, nr) in enumerate(ecg):
            nc.tensor.matmul(
                psEC[i][:, : nr * W], w_ecv1,
                e_in[0:65, r0 * W : (r0 + nr) * W],
                start=True, stop=True,
            )
        for i, (r0, nr) in enumerate(ecg):
            nc.scalar.activation(
                e1rep[:, r0 * W : (r0 + nr) * W], psEC[i][:, : nr * W],
                AF.Silu, bias=b_ecv1,
            )

        heat(e1rep[:, 0:64])

        # ---- pack enc slabs: group g covers e-rows 8g..8g+12
        for g in range(2):
            p0 = 64 * g
            src = e1rep[p0 : p0 + 50, 8 * g * W : (8 * g + 12) * W] \
                .rearrange("p (r c) -> p r c", c=W)
            dst = e1p3[p0 : p0 + 50, 0:12, 2 : 2 + W]
            if g == 0:
                nc.vector.tensor_copy(dst, src)
            else:
                nc.scalar.copy(dst, src)

        heat(e1p[:, 0:64])

        # ---- enc dw3/dw5, same three-engine split
        FS2 = 8 * XW                   # 544
        eacc_v0 = work.tile([128, FS2], BF16)
        eacc_v1 = work.tile([128, FS2], BF16)
        eacc_p = work.tile([128, FS2], BF16)
        etmp0 = work.tile([128, FS2], BF16)
        etmp1 = work.tile([128, FS2], BF16)
        etmp2 = work.tile([128, FS2], BF16)
        etmp3 = work.tile([128, FS2], BF16)
        etmps = [etmp0[:], etmp1[:], etmp2[:], etmp3[:]]
        psde0 = psA.tile([128, FS2 // 2], F32, tag="convps")
        psde1 = psA.tile([128, FS2 // 2], F32, tag="convps")
        dw_taps_dve(ED_DVE, [eacc_v0[:], eacc_v1[:]], e1p, w_edwp, FS2)
        dw_taps_act_pool(ED_ACT, eacc_p[:], etmps, e1p, w_edwp, FS2)
        dw_taps_pe(ED_PE, [psde0, psde1], edgs, e1p, FS2)
        heat(etmp1[:, 0:64])
        nc.vector.tensor_add(eacc_v0[:], eacc_v0[:], eacc_v1[:])
        nc.vector.tensor_add(eacc_v0[:], eacc_v0[:], eacc_p[:])
        nc.vector.tensor_add(
            eacc_v0[:, 0 : FS2 // 2], eacc_v0[:, 0 : FS2 // 2], psde0[:]
        )
        nc.vector.tensor_add(
            eacc_v0[:, FS2 // 2 : FS2], eacc_v0[:, FS2 // 2 : FS2], psde1[:]
        )
        nc.scalar.activation(e2p[:], eacc_v0[:], AF.Silu, bias=b_edwp)
        heat(e2p[:, 0:64])

        # repack both groups to base 0, 64-pitch contiguous
        for g in range(2):
            dst = e2c64[g][:].rearrange("p (r c) -> p r c", c=W)
            src = e2p3[64 * g : 64 * g + 50, 0:8, 0:W]
            if g == 0:
                nc.vector.tensor_copy(dst, src)
            else:
                nc.scalar.copy(dst, src)

        # ---- enc px (transposed output; K-split e1 + e2 + bias row),
        # emitted in two halves so softmax/backend stage A starts while the
        # t4-7 SiLU acts still run (costs two extra act-table switches on
        # Act, wins ~4us of backend start time)
        def epx_half(ts):
            for t in ts:
                g, lr = divmod(t, 4)
                ps = psA.tile([128, 100], F32, tag="convps", name=f"psET{t}")
                nc.tensor.matmul(
                    ps[:], e1rep[0:51, (2 * t + 2) * W : (2 * t + 4) * W],
                    w_epxA, start=True, stop=False,
                )
                nc.tensor.matmul(
                    ps[:], e2c64[g][:, 2 * lr * W : (2 * lr + 2) * W],
                    w_epxB, start=False, stop=True,
                )
                nc.scalar.activation(
                    ET[:, 100 * t : 100 * t + 100], ps[:], AF.Silu
                )

        # ---- softmax over 25 taps (no max-subtraction)
        # ET channel e within t is (k, s) raster: e = 4k + s
        expw = work.tile([128, 800], BF16)         # exp, [t][s][k] bf16
        ET_tsk = ET[:].rearrange("p (t k s) -> p t s k", t=8, k=25, s=4)
        exp4 = expw[:].rearrange("p (t s k) -> p t s k", t=8, s=4)
        S3 = S[:].rearrange("p (t s) -> p t s", s=4)
        R3 = R[:].rearrange("p (t s) -> p t s", s=4)
        wcat4 = wcat[:].rearrange("p (t s k) -> p t s k", t=8, s=4)

        def fence(lo, hi):
            # reads one element of each ET act range [lo, hi) and writes
            # inside that exp's output, so Tile cannot hoist the Exp (and
            # its act-table load) above those SiLU acts
            fsrc = ET[:, 100 * lo : 100 * hi] \
                .rearrange("p (t e) -> p t e", e=100)[:, :, 99]
            nc.scalar.copy(expw[:, 400 - (hi - lo) : 400], fsrc)

        def softmax_stage(lo, hi):
            nc.scalar.activation(exp4[:, lo:hi], ET_tsk[:, lo:hi], AF.Exp)
            nc.vector.tensor_reduce(S3[:, lo:hi], exp4[:, lo:hi], AX.X, OP.add)
            nc.vector.reciprocal(R[:, 4 * lo : 4 * hi], S[:, 4 * lo : 4 * hi])
            R4 = (
                R3[:, lo:hi]
                .unsqueeze(-1)
                .to_broadcast((128, hi - lo, 4, 25))
            )
            nc.vector.tensor_tensor(
                wcat4[:, lo:hi], exp4[:, lo:hi], R4, OP.mult
            )

        def backend_stage(t0, dve_only):
            # repl matmuls jb-major so each repl lhsT is reused 4x; all 4
            # jb's of one t land in a single bf16 PSUM bank -> one copy +
            # one scatter per t
            pss = [psB.tile([128, 512], F32, tag="small", name=f"pss{t0}_{i}")
                   for i in range(4)]
            for jb in range(4):
                for dt in range(4):
                    t = t0 + dt
                    nc.tensor.matmul(
                        pss[dt][:, 100 * jb : 100 * jb + 100],
                        repl[:, 128 * jb : 128 * jb + 128],
                        wcat[:, 100 * t : 100 * t + 100],
                        start=True, stop=True,
                    )
            for dt in range(4):
                t = t0 + dt
                dst = dall[:, 400 * t : 400 * t + 400]
                if dve_only:
                    nc.vector.tensor_copy(dst, pss[dt][0:128, 0:400])
                else:
                    nc.scalar.copy(dst, pss[dt][0:128, 0:400])
                nc.gpsimd.local_scatter(
                    b4t[:, 480 * t : 480 * t + 480],
                    dall[:, 400 * t : 400 * t + 400],
                    sidx[0:128, 0:400],
                    channels=128, num_elems=480, num_idxs=400,
                )

        epx_half(range(0, 4))
        epx_half(range(4, 8))
        fence(0, 8)
        softmax_stage(0, 4)
        backend_stage(0, dve_only=False)
        softmax_stage(4, 8)
        backend_stage(4, dve_only=True)

        psA_cm.__exit__(None, None, None)
        psC = ctx.enter_context(tc.tile_pool(name="psC", bufs=2, space="PSUM"))
        psO = ctx.enter_context(tc.tile_pool(name="psO", bufs=2, space="PSUM"))

        for t in range(8):
            # 4 transposes land in one [120, 512] bf16 PSUM bank
            psb4 = psC.tile([120, 1024], BF16, tag="b4t")
            for jb in range(4):
                nc.tensor.transpose(
                    psb4[:, 128 * jb : 128 * jb + 128],
                    b4t[:, 480 * t + 120 * jb : 480 * t + 120 * jb + 120],
                    ident,
                )
            b4 = b4pool.tile([120, 512], BF16, tag="b4")
            if t % 2 == 0:
                nc.scalar.copy(b4[:], psb4[0:120, 0:512])
            else:
                nc.vector.tensor_copy(b4[:], psb4[0:120, 0:512])

            po = psO.tile([128, 512], F32, tag="out")
            for jb in range(4):
                B = 4 * t + jb
                nc.tensor.matmul(
                    po[:, 128 * jb : 128 * jb + 128],
                    xt[:, 128 * B : 128 * B + 128],
                    b4[:, 128 * jb : 128 * jb + 128],
                    start=True, stop=True,
                )
            # one staging copy per t: po columns are (jb, r, j) raster,
            # ostage wants (r, jb, j)
            src = po[:].rearrange("c (b r j) -> c b r j", b=4, j=32)
            dst = ostage[:, 512 * t : 512 * (t + 1)] \
                .rearrange("c (r b j) -> c b r j", b=4, j=32)
            if t % 2 == 0:
                nc.vector.tensor_copy(dst, src)
            else:
                nc.scalar.copy(dst, src)
            if t in (1, 3, 5):
                eng = nc.sync if t % 4 == 1 else nc.scalar
                eng.dma_start(
                    out_d[:, 512 * (t - 1) : 512 * (t + 1)],
                    ostage[:, 512 * (t - 1) : 512 * (t + 1)],
                )
            elif t >= 6:
                eng = nc.sync if t == 6 else nc.scalar
                eng.dma_start(
                    out_d[:, 512 * t : 512 * (t + 1)],
                    ostage[:, 512 * t : 512 * (t + 1)],
                )

    nc.compile()
    return nc


_NC_CACHE = None


def _get_nc():
    global _NC_CACHE
    if _NC_CACHE is None:
        _NC_CACHE = build_kernel()
    return _NC_CACHE


def _make_in_maps(inputs):
    X = np.asarray(inputs["X"], np.float32)
    consts = _host_consts(
        {k: np.asarray(v, np.float32) for k, v in inputs.items() if k != "X"}
    )
    in_maps = []
    for core in range(NCORES):
        xs, mrow, emask, xt = _host_shard(X, core)
        pk1 = np.zeros((128, PK1_W), nbf)
        pk1[:, 0:PA_W] = consts["_pa"]
        pk1[:, PA_W : PA_W + 512] = xs[:, 0:512]
        pk2 = np.zeros((128, PK2_W), nbf)
        pk2[:, 0:1024] = xs[:, 512:1536]
        pk2[:, 1024:1536] = consts["_repl"]
        m = {
            "packb": consts["packb"],
            "sidx": consts["sidx"],
            "pk1": pk1,
            "pk2": pk2,
            "mrow": mrow,
            "emask": emask,
            "xt": xt,
        }
        in_maps.append(m)
    return in_maps


def kernel(**inputs) -> np.ndarray:
    in_maps = _make_in_maps(inputs)
    nc = _get_nc()
    res = run_bass_kernel_spmd(nc, in_maps, core_ids=list(range(NCORES)))
    out = np.zeros((2, C, 128, 128), np.float32)
    for core in range(NCORES):
        b, ri = divmod(core, 4)
        # out dram layout: [c, (t, r, col)]
        blk = res.results[core]["out"].astype(np.float32).reshape(C, 32, 128)
        out[b, :, 32 * ri : 32 * ri + 32, :] = blk
    return out


if __name__ == "__main__":
    print("smoke build only")
    build_kernel()
    print("build ok")


# revision 21
# speedup vs baseline: 1.2243x; 1.0403x over previous
"""CARAFE + MSGConv Trainium2 kernel (8 NeuronCores, spatial x batch sharding).

out[c, i, j] = sum_{p,q} W[5p+q, i, j] * Xpad[c, i//2 + p - 2, j//2 + q - 2]
 (CARAFE taps live at source resolution; identical for both subpixel parities).

Per core: one batch element (core//4) and a 16-source-row block (core%4).
The 25-tap reassembly runs on the TensorEngine as one K=120 matmul per
(row-pair, column-quarter) block:
  out[c, n] = sum_{(u,v)} X6T[(u,v), c] * B4[(u,v), n]
where B4 is a banded matrix of softmaxed W values built at runtime with
gpsimd local_scatter (per-partition index scatter) + a PE transpose; the
X side (X6T) is static data and comes pre-transposed from the host.

v3 restructure vs v2:
- one big input DMA (x | packa | repl) + xt + 4 small ones; ident moved
  into the early pack so the diag builds overlap the DMA window
- PE warm-up burst at t=0 (8 junk matmuls on a memset scratch) so HAM
  un-throttles to K=8/8 before the first real matmul
- stationary-weight reuse ordering: cv1/px run all matmuls sharing one
  lhsT back-to-back (banked PSUM accumulation groups interleaved)
- diag tiles: comp built by DVE, enc built by gpsimd local_scatter with
  an iota index (both overlap the head phase)
- backend coarsened: 4 repl matmuls land in one [128,400] PSUM bank ->
  one cast per t; 4 transposes land in one [120,512] bf16 bank -> one
  b4 copy per t; finals land in one [128,512] bank -> one stg copy per
  t into a persistent [128,4096] staging tile; 4 contiguous out DMAs
- output DRAM layout is t-major contiguous (no strided descriptors)
"""

import sys

sys.path.insert(0, "/opt/trn_rl_repo")

from contextlib import ExitStack

import ml_dtypes
import numpy as np

import concourse.bass as bass
import concourse.tile as tile
from concourse import bacc, library_config, mybir
from concourse.bass_utils import run_bass_kernel_spmd

BF16 = mybir.dt.bfloat16
F32 = mybir.dt.float32
I16 = mybir.dt.int16
AF = mybir.ActivationFunctionType
OP = mybir.AluOpType
AX = mybir.AxisListType
nbf = ml_dtypes.bfloat16

C = 128
H = W = 64
NCORES = 8
XR = 24          # X shard rows (16 + 4 halo each side)
XW = 68          # padded width for dw slabs only
NEG = -30.0      # additive pre-activation mask; SiLU(-30) ~= -2.8e-12

# packa column layout (bf16), all DMA'd in one shot with x
PA_CV1 = 0       # w_cv1_rep   [128, 128]
PA_ID = 128      # ident       [128, 128]
PA_ONES = 256    # ones row    [1, 128]
PA_PXA = 384     # w_pxA       [32, 64]
PA_PXB = 448     # w_pxB       [32, 64]
PA_ECV = 512     # w_ecv1_rep  [65, 128] (M groups at 0:50 and 64:114)
PA_EPA = 640     # w_epxA      [51, 100]
PA_EPB = 740     # w_epxB      [50, 100]
PA_EPBIAS = 840  # b_epx_row   [1, 100]
PA_EDW = 940     # w_edwp bf16 [128, 25] (for gpsimd diag builds)
PA_W = 966

PK1_W = PA_W + 512       # pk1: [packa | x[:, 0:512]]
PK2_W = 1536             # pk2: [x[:, 512:1536] | repl]

# comp dw tap split: DVE / Act / PE
CD_DVE = list(range(0, 7))
CD_ACT = list(range(7, 12))
CD_PE = list(range(12, 25))
# enc dw tap split
ED_DVE = list(range(0, 7))
ED_ACT = list(range(7, 11))
ED_PE = list(range(11, 25))


# ======================================================================
# host-side parameter prep
# ======================================================================

def _fold_1x1(w, s):
    return (w[:, :, 0, 0] * s[:, None]).T.copy()


def _dw_taps(w, s, k):
    ch = w.shape[0]
    out = np.zeros((ch, 25), np.float32)
    off = (5 - k) // 2
    for ty in range(k):
        for tx in range(k):
            out[:, 5 * (ty + off) + (tx + off)] = w[:, 0, ty, tx] * s
    return out


def _host_consts(inputs):
    d = {}
    w_cv1 = _fold_1x1(inputs["comp_cv1_w"], inputs["comp_cv1_s"])   # [128, 32]
    b_cv1 = inputs["comp_cv1_b"]                                    # [32]
    w3 = _dw_taps(inputs["comp_dw3_w"], inputs["comp_dw3_s"], 3)
    w5 = _dw_taps(inputs["comp_dw5_w"], inputs["comp_dw5_s"], 5)
    w_dwp = np.tile(np.concatenate([w3, w5], 0), (4, 1))            # [128, 25]
    b_dwp = np.tile(
        np.concatenate([inputs["comp_dw3_b"], inputs["comp_dw5_b"]]), 4
    ).reshape(128, 1)
    w_px = _fold_1x1(inputs["comp_px_w"], inputs["comp_px_s"])      # [64, 64]
    b_px = inputs["comp_px_b"].reshape(64, 1)
    we = _fold_1x1(inputs["enc_cv1_w"], inputs["enc_cv1_s"])        # [64, 50]
    w_ecv1 = np.concatenate([we, np.ones((1, 50), np.float32)], 0)  # [65, 50]
    b_ecv1 = inputs["enc_cv1_b"]                                    # [50]
    e3 = _dw_taps(inputs["enc_dw3_w"], inputs["enc_dw3_s"], 3)
    e5 = _dw_taps(inputs["enc_dw5_w"], inputs["enc_dw5_s"], 5)
    w_edw50 = np.concatenate([e3, e5], 0)                           # [50, 25]
    b_edw50 = np.concatenate(
        [inputs["enc_dw3_b"], inputs["enc_dw5_b"]]
    ).reshape(50, 1)
    wpx_e = _fold_1x1(inputs["enc_px_w"], inputs["enc_px_s"])       # [100, 100]
    b_epx = inputs["enc_px_b"].reshape(1, 100)

    w_edwp = np.zeros((128, 25), np.float32)
    w_edwp[0:50] = w_edw50
    w_edwp[64:114] = w_edw50

    pa = np.zeros((128, PA_W), np.float32)
    pa[:, PA_CV1 : PA_CV1 + 128] = np.tile(w_cv1, (1, 4))
    pa[:, PA_ID : PA_ID + 128] = np.eye(128)
    pa[0:1, PA_ONES : PA_ONES + 128] = 1.0
    pa[0:32, PA_PXA : PA_PXA + 64] = w_px[0:32]
    pa[0:32, PA_PXB : PA_PXB + 64] = w_px[32:64]
    pa[0:65, PA_ECV : PA_ECV + 50] = w_ecv1
    pa[0:65, PA_ECV + 64 : PA_ECV + 114] = w_ecv1
    pa[0:50, PA_EPA : PA_EPA + 100] = wpx_e[0:50]
    pa[50:51, PA_EPA : PA_EPA + 100] = b_epx
    pa[0:50, PA_EPB : PA_EPB + 100] = wpx_e[50:100]
    pa[0:1, PA_EPBIAS : PA_EPBIAS + 100] = b_epx
    pa[:, PA_EDW : PA_EDW + 25] = w_edwp
    d["_pa"] = pa.astype(nbf)

    pb = np.zeros((128, 56), np.float32)
    pb[:, 0:25] = w_dwp
    pb[:, 25:26] = b_dwp
    pb[0:50, 26:51] = w_edw50
    pb[64:114, 26:51] = w_edw50
    pb[0:50, 51:52] = b_edw50
    pb[64:114, 51:52] = b_edw50
    pb[:, 52:53] = np.tile(b_cv1, 4).reshape(128, 1)
    pb[0:64, 53:54] = b_px
    pb[0:50, 54:55] = b_ecv1.reshape(50, 1)
    pb[64:114, 54:55] = b_ecv1.reshape(50, 1)
    # SiLU(1.2784645) = 1: the ecv1 act writes a ones row at partition 50,
    # which carries the enc-px bias via K=51 on the e1-part matmul
    pb[50:51, 54:55] = 1.2784645427610737
    d["packb"] = pb

    # repl [128, 4*128]: lhsT for the W row-replication matmul
    # n raster within a block: n = 32*(2*yl+dy) + (2*xl+dx)
    rp = np.zeros((128, 512), np.float32)
    for jb in range(4):
        for n in range(128):
            rho, j = divmod(n, 32)
            yl, xl = rho // 2, j // 2
            rp[64 * yl + 16 * jb + xl, 128 * jb + n] = 1.0
    d["_repl"] = rp.astype(nbf)

    # sidx [128, 400+2] int16; cols 0:400 = scatter map (4 blocks per t),
    # cols 400:402 = iota idx for the gpsimd diag builds.
    si = np.full((128, 402), -1, np.int16)
    for n in range(128):
        rho, j = divmod(n, 32)
        yl, dy = divmod(rho, 2)
        xl, dx = divmod(j, 2)
        sn = 2 * dy + dx
        for jb in range(4):
            for cp in range(100):
                sc, k = divmod(cp, 25)
                if sc != sn:
                    continue
                p, q = divmod(k, 5)
                if not (0 <= 16 * jb + xl + q - 2 < 64):
                    continue
                si[n, 100 * jb + cp] = 120 * jb + 20 * (yl + p) + (xl + q)
        si[n, 400] = n
    d["sidx"] = si
    return d


def _host_shard(X, core):
    b, ri = divmod(core, 4)
    r0 = 16 * ri - 4
    xs = np.zeros((C, XR, W), np.float32)
    lo, hi = max(0, r0), min(H, r0 + XR)
    xs[:, lo - r0 : hi - r0, :] = X[b, :, lo:hi, :]
    mrow = np.zeros((1, XR, W), np.float32)
    for r in range(XR):
        if not (0 <= r0 + r < H):
            mrow[0, r, :] = NEG
    emask = np.zeros((1, 20, W), np.float32)
    for r in range(20):
        if not (0 <= (16 * ri - 2) + r < H):
            emask[0, r, :] = NEG
    xsb = xs.astype(nbf)
    # pre-transposed X slabs, one [120, 128] per block (column-padded)
    xsp = np.zeros((C, XR, XW), nbf)
    xsp[:, :, 2 : 2 + W] = xsb
    xt = np.zeros((120, 32 * 128), nbf)
    for B in range(32):
        t, jb = divmod(B, 4)
        slab = xsp[:, 2 * t + 2 : 2 * t + 8, 16 * jb : 16 * jb + 20]
        xt[:, 128 * B : 128 * B + 128] = slab.reshape(C, 120).T
    return (
        xsb.reshape(C, XR * W),
        mrow.reshape(1, XR * W).astype(nbf),
        emask.reshape(1, 20 * W).astype(nbf),
        xt,
    )


# ======================================================================
# device kernel
# ======================================================================

def build_kernel():
    nc = bacc.Bacc(
        "TRN2",
        target_bir_lowering=False,
        debug=False,
        enable_asserts=False,
        num_devices=NCORES,
    )

    def din(name, shape, dt):
        return nc.dram_tensor(name, list(shape), dt, kind="ExternalInput").ap()

    pk1_d = din("pk1", (128, PK1_W), BF16)
    pk2_d = din("pk2", (128, PK2_W), BF16)
    xt_d = din("xt", (120, 32 * 128), BF16)
    mrow_d = din("mrow", (1, XR * W), BF16)
    emask_d = din("emask", (1, 20 * W), BF16)
    packb_d = din("packb", (128, 56), F32)
    sidx_d = din("sidx", (128, 402), I16)
    # out layout: [c, (t, r, col)] -- t-major contiguous
    out_d = nc.dram_tensor("out", [128, 32 * 128], BF16, kind="ExternalOutput").ap()

    with tile.TileContext(nc) as tc, ExitStack() as ctx:
        cpool = ctx.enter_context(tc.tile_pool(name="consts", bufs=1))
        work = ctx.enter_context(tc.tile_pool(name="work", bufs=1))
        psB = ctx.enter_context(tc.tile_pool(name="psB", bufs=4, space="PSUM"))
        b4pool = ctx.enter_context(tc.tile_pool(name="b4p", bufs=3))
        psA_cm = tc.tile_pool(name="psA", bufs=4, space="PSUM")
        psA = psA_cm.__enter__()

        # ---- constant / input tiles
        pk1 = cpool.tile([128, PK1_W], BF16, tag="pk1")
        pk2 = cpool.tile([128, PK2_W], BF16, tag="pk2")
        packb = cpool.tile([128, 56], F32, tag="packb")
        xt = cpool.tile([120, 32 * 128], BF16, tag="xt")
        mrow = cpool.tile([1, XR * W], BF16, tag="mrow")
        sidx = cpool.tile([128, 402], I16, tag="sidx")

        packa = pk1[0:128, 0:PA_W]
        xb0 = pk1[0:128, PA_W : PA_W + 512]
        repl = pk2[0:128, 1024:1536]
        xbs = [xb0, pk2[0:128, 0:512], pk2[0:128, 512:1024]]

        w_cv1 = packa[0:128, PA_CV1 : PA_CV1 + 128]
        ident = packa[0:128, PA_ID : PA_ID + 128]
        ones128 = packa[0:1, PA_ONES : PA_ONES + 128]
        w_pxA = packa[0:32, PA_PXA : PA_PXA + 64]
        w_pxB = packa[0:32, PA_PXB : PA_PXB + 64]
        w_ecv1 = packa[0:65, PA_ECV : PA_ECV + 128]
        w_epxA = packa[0:51, PA_EPA : PA_EPA + 100]
        w_epxB = packa[0:50, PA_EPB : PA_EPB + 100]
        b_epx = packa[0:1, PA_EPBIAS : PA_EPBIAS + 100]
        w_edwb = packa[0:128, PA_EDW : PA_EDW + 26]
        w_dwp = packb[0:128, 0:25]
        b_dwp = packb[0:128, 25:26]
        w_edwp = packb[0:128, 26:51]
        b_edwp = packb[0:128, 51:52]
        b_cv1 = packb[0:128, 52:53]
        b_px = packb[0:64, 53:54]
        b_ecv1 = packb[0:128, 54:55]
        iota2 = sidx[0:128, 400:402]

        # ---- PE warm-up: memset scratch, then 8 junk matmuls so the HAM
        # clock gate reaches K=8/8 before the first real matmul (~3.4us of
        # sustained PE activity required; this overlaps the input DMAs)
        scr = work.tile([128, 512], BF16, tag="scr")
        scrf = work.tile([16, 4], F32, tag="scrf")
        psW = psA.tile([128, 512], F32, tag="convps")
        nc.vector.memset(scr[:], 0.0)
        nc.vector.memset(scrf[:], 0.0)
        for i in range(10):
            nc.tensor.matmul(
                psW[:], scr[:, 0:128], scr[:], start=True, stop=True
            )
        # preload the SiLU activation table off the critical path
        warm = work.tile([16, 16], BF16, tag="warm")
        nc.scalar.activation(warm[0:16, 0:2], scrf[0:16, 0:2], AF.Silu)

        # ---- queue the DMAs (per-engine FIFO order = priority order)
        nc.sync.dma_start(pk1[:], pk1_d)
        nc.sync.dma_start(pk2[:], pk2_d)
        nc.scalar.dma_start(mrow[:], mrow_d)
        nc.scalar.dma_start(packb[:], packb_d)
        nc.scalar.dma_start(sidx[:], sidx_d)
        for chk in range(2):
            nc.sync.dma_start(
                xt[:, 2048 * chk : 2048 * (chk + 1)],
                xt_d[:, 2048 * chk : 2048 * (chk + 1)],
            )

        psH = psB.tile([128, 512], F32, tag="small", name="psHeat")

        def heat(src_ap, kparts=128):
            nc.tensor.matmul(
                psH[0:32, 0:64], scr[0:kparts, 0:32], src_ap,
                start=True, stop=True,
            )

        # ---- persistent working tensors
        x1rep = work.tile([128, XR * W], BF16)     # cv1 out, 4x M-replicated
        x1p = work.tile([128, 9 * XW + 8], BF16)   # packed x1 (68-pitch)
        x2p = work.tile([128, 5 * XW], BF16)       # comp dw out slabs
        x2call = work.tile([32, 20 * W], BF16)     # dw out, rebased to 0:32
        e_in = work.tile([65, 20 * W], BF16)       # px out + mask row
        e1rep = work.tile([128, 20 * W], BF16)     # enc cv1 out (0:50, 64:114)
        e1p = work.tile([128, 12 * XW + 8], BF16)  # packed enc x1 (68-pitch)
        e2p = work.tile([128, 8 * XW], BF16)       # enc dw out slabs
        e2c64a = work.tile([50, 8 * W], BF16)      # enc dw out, 64-pitch
        e2c64b = work.tile([50, 8 * W], BF16)
        e2c64 = [e2c64a, e2c64b]
        ET = work.tile([128, 800], F32)            # enc px out (transposed)
        expv = work.tile([128, 800], F32)          # exp, [t][s][k]
        S = work.tile([128, 32], F32)              # [t][s]
        R = work.tile([128, 32], F32)
        wcat = work.tile([128, 800], BF16)         # [t][s][k]
        dall = work.tile([128, 3200], BF16)
        b4t = work.tile([128, 8 * 480], BF16)
        ostage = work.tile([128, 4096], BF16)      # [t, r, col] staging

        emask_t = e_in[64:65, :]
        nc.scalar.dma_start(emask_t, emask_d)

        x1p3 = x1p[:, 0 : 9 * XW].rearrange("p (r c) -> p r c", c=XW)
        x2p3 = x2p[:].rearrange("p (r c) -> p r c", c=XW)
        e1p3 = e1p[:, 0 : 12 * XW].rearrange("p (r c) -> p r c", c=XW)
        e2p3 = e2p[:].rearrange("p (r c) -> p r c", c=XW)

        # zero the slab tiles once (pad columns + unused partition rows)
        nc.vector.memset(x1p[:], 0.0)
        nc.vector.memset(e1p[:], 0.0)

        # ---- diag tiles for the PE dw lanes, built during the DMA window:
        # comp on DVE (tensor_scalar on ident), enc on gpsimd (local_scatter
        # with an iota index -- zero DVE cost, gpsimd is idle here)
        cdgs = []
        for i, t in enumerate(CD_PE):
            dgc = work.tile([128, 128], BF16, tag=f"dgc{i}")
            nc.vector.tensor_scalar(
                dgc[:], ident, w_dwp[:, t : t + 1], None, OP.mult
            )
            cdgs.append(dgc)
        edgs = []
        for i, t in enumerate(ED_PE):
            dge = work.tile([128, 128], BF16, tag=f"dge{i}")
            nc.gpsimd.local_scatter(
                dge[:], w_edwb[0:128, t : t + 2], iota2,
                channels=128, num_elems=128, num_idxs=2,
            )
            edgs.append(dge)

        # ---- comp cv1: 1x1 conv 128->32, M-replicated 4x (+ SiLU + mask)
        # stationary-weight-major order so the PE reuses each ldw 3x
        for ch in range(3):
            ps = psA.tile([128, 512], F32, tag="convps", name=f"psCV{ch}")
            nc.tensor.matmul(ps[:], w_cv1, xbs[ch], start=True, stop=False)
            nc.tensor.matmul(
                ps[:], ones128, mrow[:, 512 * ch : 512 * (ch + 1)],
                start=False, stop=True,
            )
            nc.scalar.activation(
                x1rep[:, 512 * ch : 512 * (ch + 1)], ps[:],
                AF.Silu, bias=b_cv1,
            )

        heat(x1rep[:, 0:64])

        # ---- pack x1 slabs: group g covers x1 rows 5g..5g+9 (engine copies)
        for g in range(4):
            src = x1rep[32 * g : 32 * g + 32, 5 * g * W : (5 * g + 9) * W] \
                .rearrange("p (r c) -> p r c", c=W)
            dst = x1p3[32 * g : 32 * g + 32, 0:9, 2 : 2 + W]
            if g % 2 == 0:
                nc.vector.tensor_copy(dst, src)
            else:
                nc.scalar.copy(dst, src)

        heat(x1p[:, 0:64])

        # ---- comp dw3/dw5 (unified 5x5 taps), split across engines
        FS = 5 * XW                    # 340
        acc_v0 = work.tile([128, FS], BF16)
        acc_v1 = work.tile([128, FS], BF16)
        acc_p = work.tile([128, FS], BF16)
        tmp0 = work.tile([128, FS], BF16)
        tmp1 = work.tile([128, FS], BF16)
        tmp2 = work.tile([128, FS], BF16)
        tmp3 = work.tile([128, FS], BF16)

        def dw_taps_dve(taps, accs, src, wcol, fs):
            for i, t in enumerate(taps):
                ty, tx = divmod(t, 5)
                sv = src[:, ty * XW + tx : ty * XW + tx + fs]
                av = accs[i % 2]
                if i < 2:
                    nc.vector.tensor_scalar(av, sv, wcol[:, t : t + 1], None, OP.mult)
                else:
                    nc.vector.scalar_tensor_tensor(
                        av, sv, wcol[:, t : t + 1], av, OP.mult, OP.add
                    )

        def dw_taps_pe(taps, psds, dgs, src, fs):
            # one diag tile per tap (PE may dedupe weight loads by address:
            # rewriting a ping-pong tile mid-group used stale weights on HW);
            # accumulation groups run sequentially, never interleaved
            nsp = len(psds)
            h = fs // nsp
            for j, psd in enumerate(psds):
                o0 = j * h
                n = h if j < nsp - 1 else fs - j * h
                for i, t in enumerate(taps):
                    ty, tx = divmod(t, 5)
                    o = ty * XW + tx + o0
                    nc.tensor.matmul(
                        psd[:, 0:n], dgs[i][:], src[:, o : o + n],
                        start=(i == 0), stop=(i == len(taps) - 1),
                    )

        def dw_taps_act_pool(taps, accp, tmps, src, wcol, fs):
            # Act scales the taps; DVE folds them in with cheap 2x-mode adds
            for i, t in enumerate(taps):
                ty, tx = divmod(t, 5)
                sv = src[:, ty * XW + tx : ty * XW + tx + fs]
                tmp = tmps[i % len(tmps)]
                nc.scalar.activation(tmp, sv, AF.Copy, scale=wcol[:, t : t + 1])
                if i == 1:
                    nc.vector.tensor_tensor(accp, tmps[0], tmps[1], OP.add)
                elif i >= 2:
                    nc.vector.tensor_tensor(accp, accp, tmp, OP.add)

        ctmps = [tmp0[:], tmp1[:], tmp2[:], tmp3[:]]
        psdc = psA.tile([128, FS], F32, tag="convps")
        dw_taps_dve(CD_DVE, [acc_v0[:], acc_v1[:]], x1p, w_dwp, FS)
        dw_taps_act_pool(CD_ACT, acc_p[:], ctmps, x1p, w_dwp, FS)
        dw_taps_pe(CD_PE, [psdc], cdgs, x1p, FS)
        heat(tmp1[:, 0:64])
        nc.vector.tensor_add(acc_v0[:], acc_v0[:], acc_v1[:])
        nc.vector.tensor_add(acc_v0[:], acc_v0[:], acc_p[:])
        nc.vector.tensor_add(acc_v0[:], acc_v0[:], psdc[:])
        nc.scalar.activation(x2p[:], acc_v0[:], AF.Silu, bias=b_dwp)
        heat(x2p[:, 0:64])

        # rebase the dw output slabs to partitions 0:32, 64-pitch contiguous
        for g in range(4):
            dst = x2call[0:32, 5 * g * W : (5 * g + 5) * W] \
                .rearrange("p (r c) -> p r c", c=W)
            src = x2p3[32 * g : 32 * g + 32, 0:5, 0:W]
            if g % 2 == 0:
                nc.vector.tensor_copy(dst, src)
            else:
                nc.scalar.copy(dst, src)

        # ---- comp px: 1x1 conv 64->64, K-split, stationary-major order
        pxg = ((0, 7), (7, 7), (14, 6))
        psPX = [psA.tile([64, nr * W], F32, tag="convps", name=f"psPX{i}")
                for i, (_, nr) in enumerate(pxg)]
        for i, (r0, nr) in enumerate(pxg):
            nc.tensor.matmul(
                psPX[i][:], w_pxA,
                x1rep[0:32, (r0 + 2) * W : (r0 + 2 + nr) * W],
                start=True, stop=False,
            )
        for i, (r0, nr) in enumerate(pxg):
            nc.tensor.matmul(
                psPX[i][:], w_pxB,
                x2call[0:32, r0 * W : (r0 + nr) * W],
                start=False, stop=True,
            )
        for i, (r0, nr) in enumerate(pxg):
            nc.scalar.activation(
                e_in[0:64, r0 * W : (r0 + nr) * W], psPX[i][:],
                AF.Silu, bias=b_px,
            )
            heat(e_in[0:64, r0 * W : r0 * W + 64], kparts=64)

        heat(e_in[0:64, 0:64], kparts=64)

        # ---- enc cv1: 1x1 conv 64->50, M-replicated 2x (mask rides K=65)
        ecg = ((0, 8), (8, 8), (16, 4))
        psEC = [psA.tile([128, 512], F32, tag="convps", name=f"psEC{i}")
                for i in range(len(ecg))]
        for i, (r0, nr) in enumerate(ecg):
            nc.tensor.matmul(
                psEC[i][:, : nr * W], w_ecv1,
                e_in[0:65, r0 * W : (r0 + nr) * W],
                start=True, stop=True,
            )
        for i, (r0, nr) in enumerate(ecg):
            nc.scalar.activation(
                e1rep[:, r0 * W : (r0 + nr) * W], psEC[i][:, : nr * W],
                AF.Silu, bias=b_ecv1,
            )

        heat(e1rep[:, 0:64])

        # ---- pack enc slabs: group g covers e-rows 8g..8g+12
        for g in range(2):
            p0 = 64 * g
            src = e1rep[p0 : p0 + 50, 8 * g * W : (8 * g + 12) * W] \
                .rearrange("p (r c) -> p r c", c=W)
            dst = e1p3[p0 : p0 + 50, 0:12, 2 : 2 + W]
            if g == 0:
                nc.vector.tensor_copy(dst, src)
            else:
                nc.scalar.copy(dst, src)

        heat(e1p[:, 0:64])

        # ---- enc dw3/dw5, same three-engine split
        FS2 = 8 * XW                   # 544
        eacc_v0 = work.tile([128, FS2], BF16)
        eacc_v1 = work.tile([128, FS2], BF16)
        eacc_p = work.tile([128, FS2], BF16)
        etmp0 = work.tile([128, FS2], BF16)
        etmp1 = work.tile([128, FS2], BF16)
        etmp2 = work.tile([128, FS2], BF16)
        etmp3 = work.tile([128, FS2], BF16)
        etmps = [etmp0[:], etmp1[:], etmp2[:], etmp3[:]]
        psde0 = psA.tile([128, FS2 // 2], F32, tag="convps")
        psde1 = psA.tile([128, FS2 // 2], F32, tag="convps")
        dw_taps_dve(ED_DVE, [eacc_v0[:], eacc_v1[:]], e1p, w_edwp, FS2)
        dw_taps_act_pool(ED_ACT, eacc_p[:], etmps, e1p, w_edwp, FS2)
        dw_taps_pe(ED_PE, [psde0, psde1], edgs, e1p, FS2)
        heat(etmp1[:, 0:64])
        nc.vector.tensor_add(eacc_v0[:], eacc_v0[:], eacc_v1[:])
        nc.vector.tensor_add(eacc_v0[:], eacc_v0[:], eacc_p[:])
        nc.vector.tensor_add(
            eacc_v0[:, 0 : FS2 // 2], eacc_v0[:, 0 : FS2 // 2], psde0[:]
        )
        nc.vector.tensor_add(
            eacc_v0[:, FS2 // 2 : FS2], eacc_v0[:, FS2 // 2 : FS2], psde1[:]
        )
        nc.scalar.activation(e2p[:], eacc_v0[:], AF.Silu, bias=b_edwp)
        heat(e2p[:, 0:64])

        # repack both groups to base 0, 64-pitch contiguous
        for g in range(2):
            dst = e2c64[g][:].rearrange("p (r c) -> p r c", c=W)
            src = e2p3[64 * g : 64 * g + 50, 0:8, 0:W]
            if g == 0:
                nc.vector.tensor_copy(dst, src)
            else:
                nc.scalar.copy(dst, src)

        # ---- enc px (transposed output; K-split e1 + e2 + bias row),
        # emitted in two halves so softmax/backend stage A starts while the
        # t4-7 SiLU acts still run (costs two extra act-table switches on
        # Act, wins ~4us of backend start time)
        def epx_half(ts):
            for t in ts:
                g, lr = divmod(t, 4)
                ps = psA.tile([128, 100], F32, tag="convps", name=f"psET{t}")
                nc.tensor.matmul(
                    ps[:], e1rep[0:51, (2 * t + 2) * W : (2 * t + 4) * W],
                    w_epxA, start=True, stop=False,
                )
                nc.tensor.matmul(
                    ps[:], e2c64[g][:, 2 * lr * W : (2 * lr + 2) * W],
                    w_epxB, start=False, stop=True,
                )
                nc.scalar.activation(
                    ET[:, 100 * t : 100 * t + 100], ps[:], AF.Silu
                )

        # ---- softmax over 25 taps (no max-subtraction)
        # ET channel e within t is (k, s) raster: e = 4k + s
        expw = work.tile([128, 800], BF16)         # exp, [t][s][k] bf16
        ET_tsk = ET[:].rearrange("p (t k s) -> p t s k", t=8, k=25, s=4)
        exp4 = expw[:].rearrange("p (t s k) -> p t s k", t=8, s=4)
        S3 = S[:].rearrange("p (t s) -> p t s", s=4)
        R3 = R[:].rearrange("p (t s) -> p t s", s=4)
        wcat4 = wcat[:].rearrange("p (t s k) -> p t s k", t=8, s=4)

        def fence(lo, hi):
            # reads one element of each ET act range [lo, hi) and writes
            # inside that exp's output, so Tile cannot hoist the Exp (and
            # its act-table load) above those SiLU acts
            fsrc = ET[:, 100 * lo : 100 * hi] \
                .rearrange("p (t e) -> p t e", e=100)[:, :, 99]
            nc.scalar.copy(expw[:, 400 - (hi - lo) : 400], fsrc)

        def softmax_stage(lo, hi):
            nc.scalar.activation(exp4[:, lo:hi], ET_tsk[:, lo:hi], AF.Exp)
            nc.vector.tensor_reduce(S3[:, lo:hi], exp4[:, lo:hi], AX.X, OP.add)
            nc.vector.reciprocal(R[:, 4 * lo : 4 * hi], S[:, 4 * lo : 4 * hi])
            R4 = (
                R3[:, lo:hi]
                .unsqueeze(-1)
                .to_broadcast((128, hi - lo, 4, 25))
            )
            nc.vector.tensor_tensor(
                wcat4[:, lo:hi], exp4[:, lo:hi], R4, OP.mult
            )

        def backend_stage(t0, dve_only):
            # repl matmuls jb-major so each repl lhsT is reused 4x; all 4
            # jb's of one t land in a single bf16 PSUM bank -> one copy +
            # one scatter per t
            pss = [psB.tile([128, 512], F32, tag="small", name=f"pss{t0}_{i}")
                   for i in range(4)]
            # t0's four matmuls first so its scatter starts asap, then
            # jb-major over the remaining t's for stationary-weight reuse
            for jb in range(4):
                nc.tensor.matmul(
                    pss[0][:, 100 * jb : 100 * jb + 100],
                    repl[:, 128 * jb : 128 * jb + 128],
                    wcat[:, 100 * t0 : 100 * t0 + 100],
                    start=True, stop=True,
                )
            for jb in range(4):
                for dt in range(1, 4):
                    t = t0 + dt
                    nc.tensor.matmul(
                        pss[dt][:, 100 * jb : 100 * jb + 100],
                        repl[:, 128 * jb : 128 * jb + 128],
                        wcat[:, 100 * t : 100 * t + 100],
                        start=True, stop=True,
                    )
            for dt in range(4):
                t = t0 + dt
                dst = dall[:, 400 * t : 400 * t + 400]
                if dve_only or dt == 0:
                    nc.vector.tensor_copy(dst, pss[dt][0:128, 0:400])
                else:
                    nc.scalar.copy(dst, pss[dt][0:128, 0:400])
                nc.gpsimd.local_scatter(
                    b4t[:, 480 * t : 480 * t + 480],
                    dall[:, 400 * t : 400 * t + 400],
                    sidx[0:128, 0:400],
                    channels=128, num_elems=480, num_idxs=400,
                )

        epx_half(range(0, 4))
        epx_half(range(4, 8))
        fence(0, 8)
        softmax_stage(0, 4)
        backend_stage(0, dve_only=False)
        softmax_stage(4, 8)
        backend_stage(4, dve_only=True)

        psA_cm.__exit__(None, None, None)
        psC = ctx.enter_context(tc.tile_pool(name="psC", bufs=2, space="PSUM"))
        psO = ctx.enter_context(tc.tile_pool(name="psO", bufs=2, space="PSUM"))

        for t in range(8):
            # 4 transposes land in one [120, 512] bf16 PSUM bank
            psb4 = psC.tile([120, 1024], BF16, tag="b4t")
            for jb in range(4):
                nc.tensor.transpose(
                    psb4[:, 128 * jb : 128 * jb + 128],
                    b4t[:, 480 * t + 120 * jb : 480 * t + 120 * jb + 120],
                    ident,
                )
            b4 = b4pool.tile([120, 512], BF16, tag="b4")
            if t % 2 == 0:
                nc.scalar.copy(b4[:], psb4[0:120, 0:512])
            else:
                nc.vector.tensor_copy(b4[:], psb4[0:120, 0:512])

            po = psO.tile([128, 512], F32, tag="out")
            for jb in range(4):
                B = 4 * t + jb
                nc.tensor.matmul(
                    po[:, 128 * jb : 128 * jb + 128],
                    xt[:, 128 * B : 128 * B + 128],
                    b4[:, 128 * jb : 128 * jb + 128],
                    start=True, stop=True,
                )
            # one staging copy per t: po columns are (jb, r, j) raster,
            # ostage wants (r, jb, j)
            src = po[:].rearrange("c (b r j) -> c b r j", b=4, j=32)
            dst = ostage[:, 512 * t : 512 * (t + 1)] \
                .rearrange("c (r b j) -> c b r j", b=4, j=32)
            if t % 2 == 1:
                nc.vector.tensor_copy(dst, src)
            else:
                nc.scalar.copy(dst, src)
            if t in (1, 3, 5):
                eng = nc.sync if t % 4 == 1 else nc.scalar
                eng.dma_start(
                    out_d[:, 512 * (t - 1) : 512 * (t + 1)],
                    ostage[:, 512 * (t - 1) : 512 * (t + 1)],
                )
            elif t >= 6:
                eng = nc.sync if t == 6 else nc.scalar
                eng.dma_start(
                    out_d[:, 512 * t : 512 * (t + 1)],
                    ostage[:, 512 * t : 512 * (t + 1)],
                )

    nc.compile()
    return nc


_NC_CACHE = None


def _get_nc():
    global _NC_CACHE
    if _NC_CACHE is None:
        _NC_CACHE = build_kernel()
    return _NC_CACHE


def _make_in_maps(inputs):
    X = np.asarray(inputs["X"], np.float32)
    consts = _host_consts(
        {k: np.asarray(v, np.float32) for k, v in inputs.items() if k != "X"}
    )
    in_maps = []
    for core in range(NCORES):
        xs, mrow, emask, xt = _host_shard(X, core)
        pk1 = np.zeros((128, PK1_W), nbf)
        pk1[:, 0:PA_W] = consts["_pa"]
        pk1[:, PA_W : PA_W + 512] = xs[:, 0:512]
        pk2 = np.zeros((128, PK2_W), nbf)
        pk2[:, 0:1024] = xs[:, 512:1536]
        pk2[:, 1024:1536] = consts["_repl"]
        m = {
            "packb": consts["packb"],
            "sidx": consts["sidx"],
            "pk1": pk1,
            "pk2": pk2,
            "mrow": mrow,
            "emask": emask,
            "xt": xt,
        }
        in_maps.append(m)
    return in_maps


def kernel(**inputs) -> np.ndarray:
    in_maps = _make_in_maps(inputs)
    nc = _get_nc()
    res = run_bass_kernel_spmd(nc, in_maps, core_ids=list(range(NCORES)))
    out = np.zeros((2, C, 128, 128), np.float32)
    for core in range(NCORES):
        b, ri = divmod(core, 4)
        # out dram layout: [c, (t, r, col)]
        blk = res.results[core]["out"].astype(np.float32).reshape(C, 32, 128)
        out[b, :, 32 * ri : 32 * ri + 32, :] = blk
    return out


if __name__ == "__main__":
    print("smoke build only")
    build_kernel()
    print("build ok")


# revision 22
# speedup vs baseline: 1.2352x; 1.0089x over previous
"""CARAFE + MSGConv Trainium2 kernel (8 NeuronCores, spatial x batch sharding).

out[c, i, j] = sum_{p,q} W[5p+q, i, j] * Xpad[c, i//2 + p - 2, j//2 + q - 2]
 (CARAFE taps live at source resolution; identical for both subpixel parities).

Per core: one batch element (core//4) and a 16-source-row block (core%4).
The 25-tap reassembly runs on the TensorEngine as one K=120 matmul per
(row-pair, column-quarter) block:
  out[c, n] = sum_{(u,v)} X6T[(u,v), c] * B4[(u,v), n]
where B4 is a banded matrix of softmaxed W values built at runtime with
gpsimd local_scatter (per-partition index scatter) + a PE transpose; the
X side (X6T) is static data and comes pre-transposed from the host.

v3 restructure vs v2:
- one big input DMA (x | packa | repl) + xt + 4 small ones; ident moved
  into the early pack so the diag builds overlap the DMA window
- PE warm-up burst at t=0 (8 junk matmuls on a memset scratch) so HAM
  un-throttles to K=8/8 before the first real matmul
- stationary-weight reuse ordering: cv1/px run all matmuls sharing one
  lhsT back-to-back (banked PSUM accumulation groups interleaved)
- diag tiles: comp built by DVE, enc built by gpsimd local_scatter with
  an iota index (both overlap the head phase)
- backend coarsened: 4 repl matmuls land in one [128,400] PSUM bank ->
  one cast per t; 4 transposes land in one [120,512] bf16 bank -> one
  b4 copy per t; finals land in one [128,512] bank -> one stg copy per
  t into a persistent [128,4096] staging tile; 4 contiguous out DMAs
- output DRAM layout is t-major contiguous (no strided descriptors)
"""

import sys

sys.path.insert(0, "/opt/trn_rl_repo")

from contextlib import ExitStack

import ml_dtypes
import numpy as np

import concourse.bass as bass
import concourse.tile as tile
from concourse import bacc, library_config, mybir
from concourse.bass_utils import run_bass_kernel_spmd

BF16 = mybir.dt.bfloat16
F32 = mybir.dt.float32
I16 = mybir.dt.int16
AF = mybir.ActivationFunctionType
OP = mybir.AluOpType
AX = mybir.AxisListType
nbf = ml_dtypes.bfloat16

C = 128
H = W = 64
NCORES = 8
XR = 24          # X shard rows (16 + 4 halo each side)
XW = 68          # padded width for dw slabs only
NEG = -30.0      # additive pre-activation mask; SiLU(-30) ~= -2.8e-12

# packa column layout (bf16), all DMA'd in one shot with x
PA_CV1 = 0       # w_cv1_rep   [128, 128]
PA_ID = 128      # ident       [128, 128]
PA_ONES = 256    # ones row    [1, 128]
PA_PXA = 384     # w_pxA       [32, 64]
PA_PXB = 448     # w_pxB       [32, 64]
PA_ECV = 512     # w_ecv1_rep  [65, 128] (M groups at 0:50 and 64:114)
PA_EPA = 640     # w_epxA      [51, 100]
PA_EPB = 740     # w_epxB      [50, 100]
PA_EPBIAS = 840  # b_epx_row   [1, 100]
PA_EDW = 940     # w_edwp bf16 [128, 25] (for gpsimd diag builds)
PA_W = 966

PK1_W = PA_W + 512       # pk1: [packa | x[:, 0:512]]
PK2_W = 1536             # pk2: [x[:, 512:1536] | repl]

# comp dw tap split: DVE / Act / PE
CD_DVE = list(range(0, 7))
CD_ACT = list(range(7, 12))
CD_PE = list(range(12, 25))
# enc dw tap split
ED_DVE = list(range(0, 7))
ED_ACT = list(range(7, 11))
ED_PE = list(range(11, 25))


# ======================================================================
# host-side parameter prep
# ======================================================================

def _fold_1x1(w, s):
    return (w[:, :, 0, 0] * s[:, None]).T.copy()


def _dw_taps(w, s, k):
    ch = w.shape[0]
    out = np.zeros((ch, 25), np.float32)
    off = (5 - k) // 2
    for ty in range(k):
        for tx in range(k):
            out[:, 5 * (ty + off) + (tx + off)] = w[:, 0, ty, tx] * s
    return out


def _host_consts(inputs):
    d = {}
    w_cv1 = _fold_1x1(inputs["comp_cv1_w"], inputs["comp_cv1_s"])   # [128, 32]
    b_cv1 = inputs["comp_cv1_b"]                                    # [32]
    w3 = _dw_taps(inputs["comp_dw3_w"], inputs["comp_dw3_s"], 3)
    w5 = _dw_taps(inputs["comp_dw5_w"], inputs["comp_dw5_s"], 5)
    w_dwp = np.tile(np.concatenate([w3, w5], 0), (4, 1))            # [128, 25]
    b_dwp = np.tile(
        np.concatenate([inputs["comp_dw3_b"], inputs["comp_dw5_b"]]), 4
    ).reshape(128, 1)
    w_px = _fold_1x1(inputs["comp_px_w"], inputs["comp_px_s"])      # [64, 64]
    b_px = inputs["comp_px_b"].reshape(64, 1)
    we = _fold_1x1(inputs["enc_cv1_w"], inputs["enc_cv1_s"])        # [64, 50]
    w_ecv1 = np.concatenate([we, np.ones((1, 50), np.float32)], 0)  # [65, 50]
    b_ecv1 = inputs["enc_cv1_b"]                                    # [50]
    e3 = _dw_taps(inputs["enc_dw3_w"], inputs["enc_dw3_s"], 3)
    e5 = _dw_taps(inputs["enc_dw5_w"], inputs["enc_dw5_s"], 5)
    w_edw50 = np.concatenate([e3, e5], 0)                           # [50, 25]
    b_edw50 = np.concatenate(
        [inputs["enc_dw3_b"], inputs["enc_dw5_b"]]
    ).reshape(50, 1)
    wpx_e = _fold_1x1(inputs["enc_px_w"], inputs["enc_px_s"])       # [100, 100]
    b_epx = inputs["enc_px_b"].reshape(1, 100)

    w_edwp = np.zeros((128, 25), np.float32)
    w_edwp[0:50] = w_edw50
    w_edwp[64:114] = w_edw50

    pa = np.zeros((128, PA_W), np.float32)
    pa[:, PA_CV1 : PA_CV1 + 128] = np.tile(w_cv1, (1, 4))
    pa[:, PA_ID : PA_ID + 128] = np.eye(128)
    pa[0:1, PA_ONES : PA_ONES + 128] = 1.0
    pa[0:32, PA_PXA : PA_PXA + 64] = w_px[0:32]
    pa[0:32, PA_PXB : PA_PXB + 64] = w_px[32:64]
    pa[0:65, PA_ECV : PA_ECV + 50] = w_ecv1
    pa[0:65, PA_ECV + 64 : PA_ECV + 114] = w_ecv1
    pa[0:50, PA_EPA : PA_EPA + 100] = wpx_e[0:50]
    pa[50:51, PA_EPA : PA_EPA + 100] = b_epx
    pa[0:50, PA_EPB : PA_EPB + 100] = wpx_e[50:100]
    pa[0:1, PA_EPBIAS : PA_EPBIAS + 100] = b_epx
    pa[:, PA_EDW : PA_EDW + 25] = w_edwp
    d["_pa"] = pa.astype(nbf)

    pb = np.zeros((128, 56), np.float32)
    pb[:, 0:25] = w_dwp
    pb[:, 25:26] = b_dwp
    pb[0:50, 26:51] = w_edw50
    pb[64:114, 26:51] = w_edw50
    pb[0:50, 51:52] = b_edw50
    pb[64:114, 51:52] = b_edw50
    pb[:, 52:53] = np.tile(b_cv1, 4).reshape(128, 1)
    pb[0:64, 53:54] = b_px
    pb[0:50, 54:55] = b_ecv1.reshape(50, 1)
    pb[64:114, 54:55] = b_ecv1.reshape(50, 1)
    # SiLU(1.2784645) = 1: the ecv1 act writes a ones row at partition 50,
    # which carries the enc-px bias via K=51 on the e1-part matmul
    pb[50:51, 54:55] = 1.2784645427610737
    d["packb"] = pb

    # repl [128, 4*128]: lhsT for the W row-replication matmul
    # n raster within a block: n = 32*(2*yl+dy) + (2*xl+dx)
    rp = np.zeros((128, 512), np.float32)
    for jb in range(4):
        for n in range(128):
            rho, j = divmod(n, 32)
            yl, xl = rho // 2, j // 2
            rp[64 * yl + 16 * jb + xl, 128 * jb + n] = 1.0
    d["_repl"] = rp.astype(nbf)

    # sidx [128, 400+2] int16; cols 0:400 = scatter map (4 blocks per t),
    # cols 400:402 = iota idx for the gpsimd diag builds.
    si = np.full((128, 402), -1, np.int16)
    for n in range(128):
        rho, j = divmod(n, 32)
        yl, dy = divmod(rho, 2)
        xl, dx = divmod(j, 2)
        sn = 2 * dy + dx
        for jb in range(4):
            for cp in range(100):
                sc, k = divmod(cp, 25)
                if sc != sn:
                    continue
                p, q = divmod(k, 5)
                if not (0 <= 16 * jb + xl + q - 2 < 64):
                    continue
                si[n, 100 * jb + cp] = 120 * jb + 20 * (yl + p) + (xl + q)
        si[n, 400] = n
    d["sidx"] = si
    return d


def _host_shard(X, core):
    b, ri = divmod(core, 4)
    r0 = 16 * ri - 4
    xs = np.zeros((C, XR, W), np.float32)
    lo, hi = max(0, r0), min(H, r0 + XR)
    xs[:, lo - r0 : hi - r0, :] = X[b, :, lo:hi, :]
    mrow = np.zeros((1, XR, W), np.float32)
    for r in range(XR):
        if not (0 <= r0 + r < H):
            mrow[0, r, :] = NEG
    emask = np.zeros((1, 20, W), np.float32)
    for r in range(20):
        if not (0 <= (16 * ri - 2) + r < H):
            emask[0, r, :] = NEG
    xsb = xs.astype(nbf)
    # pre-transposed X slabs, one [120, 128] per block (column-padded)
    xsp = np.zeros((C, XR, XW), nbf)
    xsp[:, :, 2 : 2 + W] = xsb
    xt = np.zeros((120, 32 * 128), nbf)
    for B in range(32):
        t, jb = divmod(B, 4)
        slab = xsp[:, 2 * t + 2 : 2 * t + 8, 16 * jb : 16 * jb + 20]
        xt[:, 128 * B : 128 * B + 128] = slab.reshape(C, 120).T
    return (
        xsb.reshape(C, XR * W),
        mrow.reshape(1, XR * W).astype(nbf),
        emask.reshape(1, 20 * W).astype(nbf),
        xt,
    )


# ======================================================================
# device kernel
# ======================================================================

def build_kernel():
    nc = bacc.Bacc(
        "TRN2",
        target_bir_lowering=False,
        debug=False,
        enable_asserts=False,
        num_devices=NCORES,
    )

    def din(name, shape, dt):
        return nc.dram_tensor(name, list(shape), dt, kind="ExternalInput").ap()

    pk1_d = din("pk1", (128, PK1_W), BF16)
    pk2_d = din("pk2", (128, PK2_W), BF16)
    xt_d = din("xt", (120, 32 * 128), BF16)
    mrow_d = din("mrow", (1, XR * W), BF16)
    emask_d = din("emask", (1, 20 * W), BF16)
    packb_d = din("packb", (128, 56), F32)
    sidx_d = din("sidx", (128, 402), I16)
    # out layout: [c, (t, r, col)] -- t-major contiguous
    out_d = nc.dram_tensor("out", [128, 32 * 128], BF16, kind="ExternalOutput").ap()

    with tile.TileContext(nc) as tc, ExitStack() as ctx:
        cpool = ctx.enter_context(tc.tile_pool(name="consts", bufs=1))
        work = ctx.enter_context(tc.tile_pool(name="work", bufs=1))
        psB = ctx.enter_context(tc.tile_pool(name="psB", bufs=4, space="PSUM"))
        b4pool = ctx.enter_context(tc.tile_pool(name="b4p", bufs=3))
        psA_cm = tc.tile_pool(name="psA", bufs=4, space="PSUM")
        psA = psA_cm.__enter__()

        # ---- constant / input tiles
        pk1 = cpool.tile([128, PK1_W], BF16, tag="pk1")
        pk2 = cpool.tile([128, PK2_W], BF16, tag="pk2")
        packb = cpool.tile([128, 56], F32, tag="packb")
        xt = cpool.tile([120, 32 * 128], BF16, tag="xt")
        mrow = cpool.tile([1, XR * W], BF16, tag="mrow")
        sidx = cpool.tile([128, 402], I16, tag="sidx")

        packa = pk1[0:128, 0:PA_W]
        xb0 = pk1[0:128, PA_W : PA_W + 512]
        repl = pk2[0:128, 1024:1536]
        xbs = [xb0, pk2[0:128, 0:512], pk2[0:128, 512:1024]]

        w_cv1 = packa[0:128, PA_CV1 : PA_CV1 + 128]
        ident = packa[0:128, PA_ID : PA_ID + 128]
        ones128 = packa[0:1, PA_ONES : PA_ONES + 128]
        w_pxA = packa[0:32, PA_PXA : PA_PXA + 64]
        w_pxB = packa[0:32, PA_PXB : PA_PXB + 64]
        w_ecv1 = packa[0:65, PA_ECV : PA_ECV + 128]
        w_epxA = packa[0:51, PA_EPA : PA_EPA + 100]
        w_epxB = packa[0:50, PA_EPB : PA_EPB + 100]
        b_epx = packa[0:1, PA_EPBIAS : PA_EPBIAS + 100]
        w_edwb = packa[0:128, PA_EDW : PA_EDW + 26]
        w_dwp = packb[0:128, 0:25]
        b_dwp = packb[0:128, 25:26]
        w_edwp = packb[0:128, 26:51]
        b_edwp = packb[0:128, 51:52]
        b_cv1 = packb[0:128, 52:53]
        b_px = packb[0:64, 53:54]
        b_ecv1 = packb[0:128, 54:55]
        iota2 = sidx[0:128, 400:402]

        # ---- PE warm-up: memset scratch, then 8 junk matmuls so the HAM
        # clock gate reaches K=8/8 before the first real matmul (~3.4us of
        # sustained PE activity required; this overlaps the input DMAs)
        scr = work.tile([128, 512], BF16, tag="scr")
        scrf = work.tile([16, 4], F32, tag="scrf")
        psW = psA.tile([128, 512], F32, tag="convps")
        nc.vector.memset(scr[:], 0.0)
        nc.vector.memset(scrf[:], 0.0)
        for i in range(10):
            nc.tensor.matmul(
                psW[:], scr[:, 0:128], scr[:], start=True, stop=True
            )
        # preload the SiLU activation table off the critical path
        warm = work.tile([16, 16], BF16, tag="warm")
        nc.scalar.activation(warm[0:16, 0:2], scrf[0:16, 0:2], AF.Silu)

        # ---- queue the DMAs (per-engine FIFO order = priority order)
        nc.sync.dma_start(pk1[:], pk1_d)
        nc.sync.dma_start(pk2[:], pk2_d)
        nc.scalar.dma_start(mrow[:], mrow_d)
        nc.scalar.dma_start(packb[:], packb_d)
        nc.scalar.dma_start(sidx[:], sidx_d)
        for chk in range(2):
            nc.sync.dma_start(
                xt[:, 2048 * chk : 2048 * (chk + 1)],
                xt_d[:, 2048 * chk : 2048 * (chk + 1)],
            )

        psH = psB.tile([128, 512], F32, tag="small", name="psHeat")

        def heat(src_ap, kparts=128):
            nc.tensor.matmul(
                psH[0:32, 0:64], scr[0:kparts, 0:32], src_ap,
                start=True, stop=True,
            )

        # ---- persistent working tensors
        x1rep = work.tile([128, XR * W], BF16)     # cv1 out, 4x M-replicated
        x1p = work.tile([128, 9 * XW + 8], BF16)   # packed x1 (68-pitch)
        x2p = work.tile([128, 5 * XW], BF16)       # comp dw out slabs
        x2call = work.tile([32, 20 * W], BF16)     # dw out, rebased to 0:32
        e_in = work.tile([65, 20 * W], BF16)       # px out + mask row
        e1rep = work.tile([128, 20 * W], BF16)     # enc cv1 out (0:50, 64:114)
        e1p = work.tile([128, 12 * XW + 8], BF16)  # packed enc x1 (68-pitch)
        e2p = work.tile([128, 8 * XW], BF16)       # enc dw out slabs
        e2c64a = work.tile([50, 8 * W], BF16)      # enc dw out, 64-pitch
        e2c64b = work.tile([50, 8 * W], BF16)
        e2c64 = [e2c64a, e2c64b]
        ET = work.tile([128, 800], F32)            # enc px out (transposed)
        expv = work.tile([128, 800], F32)          # exp, [t][s][k]
        S = work.tile([128, 32], F32)              # [t][s]
        R = work.tile([128, 32], F32)
        wcat = work.tile([128, 800], BF16)         # [t][s][k]
        dall = work.tile([128, 3200], BF16)
        b4t = work.tile([128, 8 * 480], BF16)
        ostage = work.tile([128, 4096], BF16)      # [t, r, col] staging

        emask_t = e_in[64:65, :]
        nc.scalar.dma_start(emask_t, emask_d)

        x1p3 = x1p[:, 0 : 9 * XW].rearrange("p (r c) -> p r c", c=XW)
        x2p3 = x2p[:].rearrange("p (r c) -> p r c", c=XW)
        e1p3 = e1p[:, 0 : 12 * XW].rearrange("p (r c) -> p r c", c=XW)
        e2p3 = e2p[:].rearrange("p (r c) -> p r c", c=XW)

        # zero the slab tiles once (pad columns + unused partition rows)
        nc.vector.memset(x1p[:], 0.0)
        nc.vector.memset(e1p[:], 0.0)

        # ---- diag tiles for the PE dw lanes, built during the DMA window:
        # comp on DVE (tensor_scalar on ident), enc on gpsimd (local_scatter
        # with an iota index -- zero DVE cost, gpsimd is idle here)
        cdgs = []
        for i, t in enumerate(CD_PE):
            dgc = work.tile([128, 128], BF16, tag=f"dgc{i}")
            nc.vector.tensor_scalar(
                dgc[:], ident, w_dwp[:, t : t + 1], None, OP.mult
            )
            cdgs.append(dgc)
        edgs = []
        for i, t in enumerate(ED_PE):
            dge = work.tile([128, 128], BF16, tag=f"dge{i}")
            nc.gpsimd.local_scatter(
                dge[:], w_edwb[0:128, t : t + 2], iota2,
                channels=128, num_elems=128, num_idxs=2,
            )
            edgs.append(dge)

        # ---- comp cv1: 1x1 conv 128->32, M-replicated 4x (+ SiLU + mask)
        # stationary-weight-major order so the PE reuses each ldw 3x
        for ch in range(3):
            ps = psA.tile([128, 512], F32, tag="convps", name=f"psCV{ch}")
            nc.tensor.matmul(ps[:], w_cv1, xbs[ch], start=True, stop=False)
            nc.tensor.matmul(
                ps[:], ones128, mrow[:, 512 * ch : 512 * (ch + 1)],
                start=False, stop=True,
            )
            nc.scalar.activation(
                x1rep[:, 512 * ch : 512 * (ch + 1)], ps[:],
                AF.Silu, bias=b_cv1,
            )

        heat(x1rep[:, 0:64])

        # ---- pack x1 slabs: group g covers x1 rows 5g..5g+9 (engine copies)
        for g in range(4):
            src = x1rep[32 * g : 32 * g + 32, 5 * g * W : (5 * g + 9) * W] \
                .rearrange("p (r c) -> p r c", c=W)
            dst = x1p3[32 * g : 32 * g + 32, 0:9, 2 : 2 + W]
            if g % 2 == 0:
                nc.vector.tensor_copy(dst, src)
            else:
                nc.scalar.copy(dst, src)

        heat(x1p[:, 0:64])

        # ---- comp px A-part (K-split over x1): the x1 side is ready as
        # soon as cv1 lands, so these run in the PE gap between cv1 and
        # the dw diag lane (keeps HAM warm with useful work)
        pxg = ((0, 7), (7, 7), (14, 6))
        psPX = [psA.tile([64, 512], F32, tag="convps", name=f"psPX{i}")
                for i in range(3)]
        for i, (r0, nr) in enumerate(pxg):
            nc.tensor.matmul(
                psPX[i][:, : nr * W], w_pxA,
                x1rep[0:32, (r0 + 2) * W : (r0 + 2 + nr) * W],
                start=True, stop=False,
            )

        # ---- comp dw3/dw5 (unified 5x5 taps), split across engines
        FS = 5 * XW                    # 340
        acc_v0 = work.tile([128, FS], BF16)
        acc_v1 = work.tile([128, FS], BF16)
        acc_p = work.tile([128, FS], BF16)
        tmp0 = work.tile([128, FS], BF16)
        tmp1 = work.tile([128, FS], BF16)
        tmp2 = work.tile([128, FS], BF16)
        tmp3 = work.tile([128, FS], BF16)

        def dw_taps_dve(taps, accs, src, wcol, fs):
            for i, t in enumerate(taps):
                ty, tx = divmod(t, 5)
                sv = src[:, ty * XW + tx : ty * XW + tx + fs]
                av = accs[i % 2]
                if i < 2:
                    nc.vector.tensor_scalar(av, sv, wcol[:, t : t + 1], None, OP.mult)
                else:
                    nc.vector.scalar_tensor_tensor(
                        av, sv, wcol[:, t : t + 1], av, OP.mult, OP.add
                    )

        def dw_taps_pe(taps, psds, dgs, src, fs):
            # one diag tile per tap (PE may dedupe weight loads by address:
            # rewriting a ping-pong tile mid-group used stale weights on HW);
            # accumulation groups run sequentially, never interleaved
            nsp = len(psds)
            h = fs // nsp
            for j, psd in enumerate(psds):
                o0 = j * h
                n = h if j < nsp - 1 else fs - j * h
                for i, t in enumerate(taps):
                    ty, tx = divmod(t, 5)
                    o = ty * XW + tx + o0
                    nc.tensor.matmul(
                        psd[:, 0:n], dgs[i][:], src[:, o : o + n],
                        start=(i == 0), stop=(i == len(taps) - 1),
                    )

        def dw_taps_act_pool(taps, accp, tmps, src, wcol, fs):
            # Act scales the taps; DVE folds them in with cheap 2x-mode adds
            for i, t in enumerate(taps):
                ty, tx = divmod(t, 5)
                sv = src[:, ty * XW + tx : ty * XW + tx + fs]
                tmp = tmps[i % len(tmps)]
                nc.scalar.activation(tmp, sv, AF.Copy, scale=wcol[:, t : t + 1])
                if i == 1:
                    nc.vector.tensor_tensor(accp, tmps[0], tmps[1], OP.add)
                elif i >= 2:
                    nc.vector.tensor_tensor(accp, accp, tmp, OP.add)

        ctmps = [tmp0[:], tmp1[:], tmp2[:], tmp3[:]]
        psdc = psA.tile([128, FS], F32, tag="convps")
        dw_taps_dve(CD_DVE, [acc_v0[:], acc_v1[:]], x1p, w_dwp, FS)
        dw_taps_act_pool(CD_ACT, acc_p[:], ctmps, x1p, w_dwp, FS)
        dw_taps_pe(CD_PE, [psdc], cdgs, x1p, FS)
        heat(tmp1[:, 0:64])
        nc.vector.tensor_add(acc_v0[:], acc_v0[:], acc_v1[:])
        nc.vector.tensor_add(acc_v0[:], acc_v0[:], acc_p[:])
        nc.vector.tensor_add(acc_v0[:], acc_v0[:], psdc[:])
        nc.scalar.activation(x2p[:], acc_v0[:], AF.Silu, bias=b_dwp)
        heat(x2p[:, 0:64])

        # rebase the dw output slabs to partitions 0:32, 64-pitch contiguous
        for g in range(4):
            dst = x2call[0:32, 5 * g * W : (5 * g + 5) * W] \
                .rearrange("p (r c) -> p r c", c=W)
            src = x2p3[32 * g : 32 * g + 32, 0:5, 0:W]
            if g % 2 == 0:
                nc.vector.tensor_copy(dst, src)
            else:
                nc.scalar.copy(dst, src)

        # ---- comp px B-part (x2 side) + SiLU
        for i, (r0, nr) in enumerate(pxg):
            nc.tensor.matmul(
                psPX[i][:, : nr * W], w_pxB,
                x2call[0:32, r0 * W : (r0 + nr) * W],
                start=False, stop=True,
            )
        for i, (r0, nr) in enumerate(pxg):
            nc.scalar.activation(
                e_in[0:64, r0 * W : (r0 + nr) * W], psPX[i][0:64, : nr * W],
                AF.Silu, bias=b_px,
            )
            heat(e_in[0:64, r0 * W : r0 * W + 64], kparts=64)

        heat(e_in[0:64, 0:64], kparts=64)

        # ---- enc cv1: 1x1 conv 64->50, M-replicated 2x (mask rides K=65)
        ecg = ((0, 8), (8, 8), (16, 4))
        psEC = [psA.tile([128, 512], F32, tag="convps", name=f"psEC{i}")
                for i in range(len(ecg))]
        for i, (r0, nr) in enumerate(ecg):
            nc.tensor.matmul(
                psEC[i][:, : nr * W], w_ecv1,
                e_in[0:65, r0 * W : (r0 + nr) * W],
                start=True, stop=True,
            )
        for i, (r0, nr) in enumerate(ecg):
            nc.scalar.activation(
                e1rep[:, r0 * W : (r0 + nr) * W], psEC[i][:, : nr * W],
                AF.Silu, bias=b_ecv1,
            )

        heat(e1rep[:, 0:64])

        # ---- pack enc slabs: group g covers e-rows 8g..8g+12
        for g in range(2):
            p0 = 64 * g
            src = e1rep[p0 : p0 + 50, 8 * g * W : (8 * g + 12) * W] \
                .rearrange("p (r c) -> p r c", c=W)
            dst = e1p3[p0 : p0 + 50, 0:12, 2 : 2 + W]
            if g == 0:
                nc.vector.tensor_copy(dst, src)
            else:
                nc.scalar.copy(dst, src)

        heat(e1p[:, 0:64])

        # ---- enc dw3/dw5, same three-engine split
        FS2 = 8 * XW                   # 544
        eacc_v0 = work.tile([128, FS2], BF16)
        eacc_v1 = work.tile([128, FS2], BF16)
        eacc_p = work.tile([128, FS2], BF16)
        etmp0 = work.tile([128, FS2], BF16)
        etmp1 = work.tile([128, FS2], BF16)
        etmp2 = work.tile([128, FS2], BF16)
        etmp3 = work.tile([128, FS2], BF16)
        etmps = [etmp0[:], etmp1[:], etmp2[:], etmp3[:]]
        psde0 = psA.tile([128, FS2 // 2], F32, tag="convps")
        psde1 = psA.tile([128, FS2 // 2], F32, tag="convps")
        dw_taps_dve(ED_DVE, [eacc_v0[:], eacc_v1[:]], e1p, w_edwp, FS2)
        dw_taps_act_pool(ED_ACT, eacc_p[:], etmps, e1p, w_edwp, FS2)
        dw_taps_pe(ED_PE, [psde0, psde1], edgs, e1p, FS2)
        heat(etmp1[:, 0:64])
        nc.vector.tensor_add(eacc_v0[:], eacc_v0[:], eacc_v1[:])
        nc.vector.tensor_add(eacc_v0[:], eacc_v0[:], eacc_p[:])
        nc.vector.tensor_add(
            eacc_v0[:, 0 : FS2 // 2], eacc_v0[:, 0 : FS2 // 2], psde0[:]
        )
        nc.vector.tensor_add(
            eacc_v0[:, FS2 // 2 : FS2], eacc_v0[:, FS2 // 2 : FS2], psde1[:]
        )
        nc.scalar.activation(e2p[:], eacc_v0[:], AF.Silu, bias=b_edwp)
        heat(e2p[:, 0:64])

        # repack both groups to base 0, 64-pitch contiguous
        for g in range(2):
            dst = e2c64[g][:].rearrange("p (r c) -> p r c", c=W)
            src = e2p3[64 * g : 64 * g + 50, 0:8, 0:W]
            if g == 0:
                nc.vector.tensor_copy(dst, src)
            else:
                nc.scalar.copy(dst, src)

        # ---- enc px (transposed output; K-split e1 + e2 + bias row),
        # emitted in two halves so softmax/backend stage A starts while the
        # t4-7 SiLU acts still run (costs two extra act-table switches on
        # Act, wins ~4us of backend start time)
        def epx_half(ts):
            # all four t's of a half land in one PSUM bank -> one wide SiLU
            h = ts[0] // 4
            ps = psA.tile([128, 512], F32, tag="convps", name=f"psET{h}")
            for t in ts:
                g, lr = divmod(t, 4)
                c0 = 100 * (t - ts[0])
                nc.tensor.matmul(
                    ps[:, c0 : c0 + 100],
                    e1rep[0:51, (2 * t + 2) * W : (2 * t + 4) * W],
                    w_epxA, start=True, stop=False,
                )
                nc.tensor.matmul(
                    ps[:, c0 : c0 + 100],
                    e2c64[g][:, 2 * lr * W : (2 * lr + 2) * W],
                    w_epxB, start=False, stop=True,
                )
            nc.scalar.activation(
                ET[:, 400 * h : 400 * h + 400], ps[0:128, 0:400], AF.Silu
            )

        # ---- softmax over 25 taps (no max-subtraction)
        # ET channel e within t is (k, s) raster: e = 4k + s
        expw = work.tile([128, 800], BF16)         # exp, [t][s][k] bf16
        ET_tsk = ET[:].rearrange("p (t k s) -> p t s k", t=8, k=25, s=4)
        exp4 = expw[:].rearrange("p (t s k) -> p t s k", t=8, s=4)
        S3 = S[:].rearrange("p (t s) -> p t s", s=4)
        R3 = R[:].rearrange("p (t s) -> p t s", s=4)
        wcat4 = wcat[:].rearrange("p (t s k) -> p t s k", t=8, s=4)

        def fence(lo, hi):
            # reads one element of each ET act range [lo, hi) and writes
            # inside that exp's output, so Tile cannot hoist the Exp (and
            # its act-table load) above those SiLU acts
            fsrc = ET[:, 100 * lo : 100 * hi] \
                .rearrange("p (t e) -> p t e", e=100)[:, :, 99]
            nc.scalar.copy(expw[:, 400 - (hi - lo) : 400], fsrc)

        def softmax_stage(lo, hi):
            nc.scalar.activation(exp4[:, lo:hi], ET_tsk[:, lo:hi], AF.Exp)
            nc.vector.tensor_reduce(S3[:, lo:hi], exp4[:, lo:hi], AX.X, OP.add)
            nc.vector.reciprocal(R[:, 4 * lo : 4 * hi], S[:, 4 * lo : 4 * hi])
            R4 = (
                R3[:, lo:hi]
                .unsqueeze(-1)
                .to_broadcast((128, hi - lo, 4, 25))
            )
            nc.vector.tensor_tensor(
                wcat4[:, lo:hi], exp4[:, lo:hi], R4, OP.mult
            )

        def backend_stage(t0, dve_only):
            # repl matmuls jb-major so each repl lhsT is reused 4x; all 4
            # jb's of one t land in a single bf16 PSUM bank -> one copy +
            # one scatter per t
            pss = [psB.tile([128, 512], F32, tag="small", name=f"pss{t0}_{i}")
                   for i in range(4)]
            # t0's four matmuls first so its scatter starts asap, then
            # jb-major over the remaining t's for stationary-weight reuse
            for jb in range(4):
                nc.tensor.matmul(
                    pss[0][:, 100 * jb : 100 * jb + 100],
                    repl[:, 128 * jb : 128 * jb + 128],
                    wcat[:, 100 * t0 : 100 * t0 + 100],
                    start=True, stop=True,
                )
            for jb in range(4):
                for dt in range(1, 4):
                    t = t0 + dt
                    nc.tensor.matmul(
                        pss[dt][:, 100 * jb : 100 * jb + 100],
                        repl[:, 128 * jb : 128 * jb + 128],
                        wcat[:, 100 * t : 100 * t + 100],
                        start=True, stop=True,
                    )
            for dt in range(4):
                t = t0 + dt
                dst = dall[:, 400 * t : 400 * t + 400]
                if dve_only or dt == 0:
                    nc.vector.tensor_copy(dst, pss[dt][0:128, 0:400])
                else:
                    nc.scalar.copy(dst, pss[dt][0:128, 0:400])
                nc.gpsimd.local_scatter(
                    b4t[:, 480 * t : 480 * t + 480],
                    dall[:, 400 * t : 400 * t + 400],
                    sidx[0:128, 0:400],
                    channels=128, num_elems=480, num_idxs=400,
                )

        epx_half(range(0, 4))
        epx_half(range(4, 8))
        fence(0, 8)
        softmax_stage(0, 4)
        backend_stage(0, dve_only=False)
        softmax_stage(4, 8)
        backend_stage(4, dve_only=True)

        psA_cm.__exit__(None, None, None)
        psC = ctx.enter_context(tc.tile_pool(name="psC", bufs=2, space="PSUM"))
        psO = ctx.enter_context(tc.tile_pool(name="psO", bufs=2, space="PSUM"))

        for t in range(8):
            # 4 transposes land in one [120, 512] bf16 PSUM bank
            psb4 = psC.tile([120, 1024], BF16, tag="b4t")
            for jb in range(4):
                nc.tensor.transpose(
                    psb4[:, 128 * jb : 128 * jb + 128],
                    b4t[:, 480 * t + 120 * jb : 480 * t + 120 * jb + 120],
                    ident,
                )
            b4 = b4pool.tile([120, 512], BF16, tag="b4")
            if t % 2 == 0:
                nc.scalar.copy(b4[:], psb4[0:120, 0:512])
            else:
                nc.vector.tensor_copy(b4[:], psb4[0:120, 0:512])

            po = psO.tile([128, 512], F32, tag="out")
            for jb in range(4):
                B = 4 * t + jb
                nc.tensor.matmul(
                    po[:, 128 * jb : 128 * jb + 128],
                    xt[:, 128 * B : 128 * B + 128],
                    b4[:, 128 * jb : 128 * jb + 128],
                    start=True, stop=True,
                )
            # one staging copy per t: po columns are (jb, r, j) raster,
            # ostage wants (r, jb, j)
            src = po[:].rearrange("c (b r j) -> c b r j", b=4, j=32)
            dst = ostage[:, 512 * t : 512 * (t + 1)] \
                .rearrange("c (r b j) -> c b r j", b=4, j=32)
            if t % 2 == 1:
                nc.vector.tensor_copy(dst, src)
            else:
                nc.scalar.copy(dst, src)
            if t in (1, 3, 5):
                eng = nc.sync if t % 4 == 1 else nc.scalar
                eng.dma_start(
                    out_d[:, 512 * (t - 1) : 512 * (t + 1)],
                    ostage[:, 512 * (t - 1) : 512 * (t + 1)],
                )
            elif t >= 6:
                eng = nc.sync if t == 6 else nc.scalar
                eng.dma_start(
                    out_d[:, 512 * t : 512 * (t + 1)],
                    ostage[:, 512 * t : 512 * (t + 1)],
                )

    nc.compile()
    return nc


_NC_CACHE = None


def _get_nc():
    global _NC_CACHE
    if _NC_CACHE is None:
        _NC_CACHE = build_kernel()
    return _NC_CACHE


def _make_in_maps(inputs):
    X = np.asarray(inputs["X"], np.float32)
    consts = _host_consts(
        {k: np.asarray(v, np.float32) for k, v in inputs.items() if k != "X"}
    )
    in_maps = []
    for core in range(NCORES):
        xs, mrow, emask, xt = _host_shard(X, core)
        pk1 = np.zeros((128, PK1_W), nbf)
        pk1[:, 0:PA_W] = consts["_pa"]
        pk1[:, PA_W : PA_W + 512] = xs[:, 0:512]
        pk2 = np.zeros((128, PK2_W), nbf)
        pk2[:, 0:1024] = xs[:, 512:1536]
        pk2[:, 1024:1536] = consts["_repl"]
        m = {
            "packb": consts["packb"],
            "sidx": consts["sidx"],
            "pk1": pk1,
            "pk2": pk2,
            "mrow": mrow,
            "emask": emask,
            "xt": xt,
        }
        in_maps.append(m)
    return in_maps


def kernel(**inputs) -> np.ndarray:
    in_maps = _make_in_maps(inputs)
    nc = _get_nc()
    res = run_bass_kernel_spmd(nc, in_maps, core_ids=list(range(NCORES)))
    out = np.zeros((2, C, 128, 128), np.float32)
    for core in range(NCORES):
        b, ri = divmod(core, 4)
        # out dram layout: [c, (t, r, col)]
        blk = res.results[core]["out"].astype(np.float32).reshape(C, 32, 128)
        out[b, :, 32 * ri : 32 * ri + 32, :] = blk
    return out


if __name__ == "__main__":
    print("smoke build only")
    build_kernel()
    print("build ok")
